# revision 7
# baseline (speedup 1.0000x reference)
"""3-layer GCN (GCNConvNet) on 8 Trainium2 NeuronCores.

Math refactor: with isd = 1/sqrt(deg+1) and self-loop edges folded in,
each GCN layer  h' = relu( D^-1/2 (A+I) D^-1/2 (h W^T + 1 b^T) )  becomes

    g      = isd**2 * relu(Q_prev)          (node-major "source features")
    P[n]   = sum_{e: dst(e)=n} g[src(e)]    (+ g[n] self term)
    Q[n]   = Waug^T @ [P[n]; sigma[n]]      (Waug = [W^T; b], sigma = row sums)
    h'     = relu(isd * Q) = isd * relu(Q)  -> g' = isd^2 * relu(Q)

so every per-edge coefficient disappears into per-node scaling and the
scatter matrices are pure one-hot.  The final layer output is isd * Q2.

Sharding: nodes split into 8 contiguous dst ranges (6250 each).  Each core
computes P for its own range over ALL edges.  Edge gathers use
nc.gpsimd.dma_gather (int16 indices) with 256B elements (64 fp16 features
padded to 128 columns).

The source table is split in two permuted half-tables so that (a) row
indices stay below 2^15 and (b) the inter-layer AllGather splits into two
independent halves:

  A: rows (src % 6250) <  3200 -> table row core(src)*3200 + local
  B: rows (src % 6250) >= 3200 -> table row core(src)*3050 + local-3200

After each layer the core's own A-half rows (dst tiles 0-24) finish first,
so the A AllGather is issued while tiles 25-48 still compute, and the next
layer's A-stream gathers start as soon as it lands -- overlapping the B
AllGather with real work on every engine.

The per-dst-tile chunk structure is derived from the actual edge data at
kernel() call time and padded to the max over the 8 cores so that all
cores run one shared NEFF (SPMD).
"""

import math
import numpy as np

NC_CORES = 8
TILE = 128
GRP_TILES = 4  # dst tiles fused per PSUM/matmul group (4*128 = 512 <= max N)
D_F = 64  # feature width of hidden layers
D_PAD = 128  # padded row width so a gather element is 256B
LO_TILES = 25  # dst tiles in the A (low) half of each core's own range
WIN = 8  # chunks per gather window (8*128 = 1024 descs = SWDGE ring limit)
NQ = 4  # SWDGE queues


# ----------------------------------------------------------------------------
# host-side graph preprocessing
# ----------------------------------------------------------------------------


def _wrap16(v):
    """[S] int -> [128, S//16] int16, index i at [i%16, i//16], replicated x8."""
    S = v.shape[0]
    assert S % 16 == 0
    w = v.reshape(S // 16, 16).T.astype(np.int16)
    return np.ascontiguousarray(np.tile(w, (8, 1)))


def _prepare(x, edge_index, W0, b0, W1, b1, W2, b2):
    x = np.asarray(x, dtype=np.float32)
    ei = np.asarray(edge_index)
    W0 = np.asarray(W0, np.float32)
    b0 = np.asarray(b0, np.float32)
    W1 = np.asarray(W1, np.float32)
    b1 = np.asarray(b1, np.float32)
    W2 = np.asarray(W2, np.float32)
    b2 = np.asarray(b2, np.float32)

    N = x.shape[0]
    assert N % NC_CORES == 0
    OWN = N // NC_CORES
    ntiles = (OWN + TILE - 1) // TILE
    LO = LO_TILES * TILE  # own-local rows in the A half
    HI = OWN - LO
    NA, NB = NC_CORES * LO, NC_CORES * HI
    assert max(NA, NB) <= 32768, "int16 gather indices"
    src = ei[0].astype(np.int64)
    dst = ei[1].astype(np.int64)

    deg = np.bincount(dst, minlength=N).astype(np.float32) + 1.0
    isd = (1.0 / np.sqrt(deg)).astype(np.float32)
    sigma = (
        np.bincount(dst, weights=isd[src].astype(np.float64), minlength=N).astype(
            np.float32
        )
        + isd
    )

    g0 = np.zeros((N, D_PAD), np.float16)
    g0[:, :D_F] = (isd[:, None] * x).astype(np.float16)
    # permuted half tables: A row = core*LO + local (local < LO)
    #                      B row = core*HI + local - LO
    loc_all = np.arange(N) % OWN
    g0A = np.ascontiguousarray(g0[loc_all < LO])  # ordered by (core, local)
    g0B = np.ascontiguousarray(g0[loc_all >= LO])

    # ---- edge bucketing: (core, tile, half) ---------------------------------
    s_core = src // OWN
    s_loc = src % OWN
    half = (s_loc >= LO).astype(np.int64)
    rowA = s_core * LO + s_loc  # valid where half==0
    rowB = s_core * HI + (s_loc - LO)  # valid where half==1
    row = np.where(half == 0, rowA, rowB)
    core = dst // OWN
    tl = (dst % OWN) // TILE
    key = (core * ntiles + tl) * 2 + half
    order = np.argsort(key, kind="stable")
    s_row = row[order]
    s_dstl = (dst % OWN) % TILE
    s_dstl = s_dstl[order]
    counts = np.bincount(key, minlength=NC_CORES * ntiles * 2).reshape(
        NC_CORES, ntiles, 2
    )
    starts = np.zeros(NC_CORES * ntiles * 2 + 1, np.int64)
    np.cumsum(counts.reshape(-1), out=starts[1:])

    # chunks per (tile, half), shared across cores
    CA = np.maximum(1, -(-counts[:, :, 0].max(axis=0) // TILE)).astype(np.int64)
    CB = np.maximum(1, -(-counts[:, :, 1].max(axis=0) // TILE)).astype(np.int64)
    # (CA/CB >= 1 keeps gather groups non-empty; pure-pad chunks are cheap)

    a_off = np.zeros(ntiles + 1, np.int64)  # chunk offsets into the A stream
    np.cumsum(CA, out=a_off[1:])
    b_off = np.zeros(ntiles + 1, np.int64)
    np.cumsum(CB, out=b_off[1:])
    chunk_base = np.zeros(ntiles + 1, np.int64)
    np.cumsum(CA + CB, out=chunk_base[1:])
    SA, SB = int(a_off[-1]) * TILE, int(b_off[-1]) * TILE
    nchunk = int(chunk_base[-1])

    per_core = []
    for c in range(NC_CORES):
        # pad slots must gather *something*; spread them over distinct rows
        # so they don't hammer one HBM line (S-col is -1 so the gathered
        # values never contribute).
        sA = np.arange(SA, dtype=np.int64) % NA
        sB = np.arange(SB, dtype=np.int64) % NB
        dstl_flat = np.full(nchunk * TILE, -1.0, np.float32)
        for t in range(ntiles):
            k = (c * ntiles + t) * 2
            lo, hi = starts[k], starts[k + 1]
            nA = hi - lo
            a_slot = a_off[t] * TILE
            sA[a_slot : a_slot + nA] = s_row[lo:hi]
            dstl_flat[chunk_base[t] * TILE : chunk_base[t] * TILE + nA] = s_dstl[
                lo:hi
            ]
            lo, hi = starts[k + 1], starts[k + 2]
            nB = hi - lo
            b_slot = b_off[t] * TILE
            sB[b_slot : b_slot + nB] = s_row[lo:hi]
            boff = (chunk_base[t] + CA[t]) * TILE
            dstl_flat[boff : boff + nB] = s_dstl[lo:hi]
        own = isd[c * OWN : (c + 1) * OWN] ** 2
        tmp = np.zeros(ntiles * TILE, np.float32)
        tmp[:OWN] = own
        isd2 = np.ascontiguousarray(tmp.reshape(ntiles, TILE).T)
        per_core.append(
            dict(
                idxA=_wrap16(sA),
                idxB=_wrap16(sB),
                dstl=np.ascontiguousarray(
                    dstl_flat.reshape(nchunk, TILE).T.astype(np.float16)
                ),
                sigma=sigma[c * OWN : (c + 1) * OWN]
                .astype(np.float16)
                .reshape(1, OWN),
                isd2=isd2,
                isdrow=isd[c * OWN : (c + 1) * OWN]
                .astype(np.float32)
                .reshape(1, OWN),
                g0ownA=np.ascontiguousarray(g0[c * OWN : c * OWN + LO]),
                g0ownB=np.ascontiguousarray(g0[c * OWN + LO : (c + 1) * OWN]),
            )
        )

    waug = []
    for W, b in ((W0, b0), (W1, b1), (W2, b2)):
        wa = np.zeros((D_F + 1, W.shape[0]), np.float16)
        wa[:D_F, :] = W.T.astype(np.float16)
        wa[D_F, :] = b.astype(np.float16)
        waug.append(wa)

    iota = np.tile(np.arange(TILE, dtype=np.float16), (TILE, 1))
    ident = np.eye(TILE, dtype=np.float16)

    meta = dict(
        N=N,
        OWN=OWN,
        LO=LO,
        HI=HI,
        NA=NA,
        NB=NB,
        ntiles=ntiles,
        CA=CA,
        CB=CB,
        a_off=a_off,
        b_off=b_off,
        chunk_base=chunk_base,
        SA=SA,
        SB=SB,
        nchunk=nchunk,
        d_out=W2.shape[0],
    )

    in_maps = []
    for c in range(NC_CORES):
        m = dict(per_core[c])
        m["g0A"] = g0A
        m["g0B"] = g0B
        m["waug0"] = waug[0]
        m["waug1"] = waug[1]
        m["waug2"] = waug[2]
        m["iota"] = iota
        m["ident"] = ident
        in_maps.append(m)
    return meta, in_maps


# ----------------------------------------------------------------------------
# device kernel
# ----------------------------------------------------------------------------


def _build(meta, stage=99, n_dev=NC_CORES):
    # stage gates for HW bisection: 1 gathers, 2 +S build, 3 +seg matmuls,
    # 4 +aug matmul, 5 +postproc/gown, 6 +collective, >=7 all three layers.
    import concourse.bacc as bacc
    import concourse.mybir as mybir
    from concourse.tile import TileContext

    f16 = mybir.dt.float16
    f32 = mybir.dt.float32
    i16 = mybir.dt.int16

    N = meta["N"]
    OWN = meta["OWN"]
    LO, HI = meta["LO"], meta["HI"]
    NA, NB = meta["NA"], meta["NB"]
    ntiles = meta["ntiles"]
    CA, CB = meta["CA"], meta["CB"]
    a_off, b_off = meta["a_off"], meta["b_off"]
    chunk_base = meta["chunk_base"]
    SA, SB, nchunk = meta["SA"], meta["SB"], meta["nchunk"]
    d_out = meta["d_out"]

    ngrp = (ntiles + GRP_TILES - 1) // GRP_TILES
    grp_tiles = [
        list(range(g * GRP_TILES, min((g + 1) * GRP_TILES, ntiles)))
        for g in range(ngrp)
    ]
    max_ch = max(
        int(chunk_base[ts[-1] + 1] - chunk_base[ts[0]]) for ts in grp_tiles
    )

    nc = bacc.Bacc("TRN2", target_bir_lowering=False, num_devices=n_dev,
                  num_swdge_queues=NQ)

    g0A_d = nc.dram_tensor("g0A", [NA, D_PAD], f16, kind="ExternalInput")
    g0B_d = nc.dram_tensor("g0B", [NB, D_PAD], f16, kind="ExternalInput")
    g0ownA_d = nc.dram_tensor("g0ownA", [LO, D_PAD], f16, kind="ExternalInput")
    g0ownB_d = nc.dram_tensor("g0ownB", [HI, D_PAD], f16, kind="ExternalInput")
    idxA_d = nc.dram_tensor("idxA", [128, SA // 16], i16, kind="ExternalInput")
    idxB_d = nc.dram_tensor("idxB", [128, SB // 16], i16, kind="ExternalInput")
    dstl_d = nc.dram_tensor("dstl", [128, nchunk], f16, kind="ExternalInput")
    waug_d = [
        nc.dram_tensor(f"waug{l}", [D_F + 1, do], f16, kind="ExternalInput")
        for l, do in enumerate([D_F, D_F, d_out])
    ]
    sigma_d = nc.dram_tensor("sigma", [1, OWN], f16, kind="ExternalInput")
    isd2_d = nc.dram_tensor("isd2", [TILE, ntiles], f32, kind="ExternalInput")
    isdrow_d = nc.dram_tensor("isdrow", [1, OWN], f32, kind="ExternalInput")
    iota_d = nc.dram_tensor("iota", [TILE, TILE], f16, kind="ExternalInput")
    ident_d = nc.dram_tensor("ident", [TILE, TILE], f16, kind="ExternalInput")
    out_d = nc.dram_tensor("out", [1, OWN], f32, kind="ExternalOutput")

    gownA_d = [nc.dram_tensor(f"gownA{l}", [LO, D_PAD], f16) for l in (1, 2)]
    gownB_d = [nc.dram_tensor(f"gownB{l}", [HI, D_PAD], f16) for l in (1, 2)]
    gfullA_d = [
        nc.dram_tensor(f"gfullA{l}", [NA, D_PAD], f16, addr_space="Shared")
        for l in (1, 2)
    ]
    gfullB_d = [
        nc.dram_tensor(f"gfullB{l}", [NB, D_PAD], f16, addr_space="Shared")
        for l in (1, 2)
    ]

    rg = [list(range(NC_CORES))]

    with TileContext(nc) as tc:
        with (
            tc.tile_pool(name="static", bufs=1) as stp,
            tc.tile_pool(name="msgs", bufs=2 * NQ) as mp,
            tc.tile_pool(name="smat", bufs=2) as sp,
            tc.tile_pool(name="gself", bufs=2) as gp,
            tc.tile_pool(name="paug", bufs=2) as pp,
            tc.tile_pool(name="qrelu", bufs=2) as qp,
            tc.tile_pool(name="gout", bufs=3) as gop,
            tc.tile_pool(name="pps", bufs=2, space="PSUM") as p_ps,
            tc.tile_pool(name="qps", bufs=2, space="PSUM") as q_ps,
            tc.tile_pool(name="tps", bufs=2, space="PSUM") as t_ps,
        ):
            # dma_gather burns one GPSIMD register per distinct num_idxs via
            # to_reg; cache by value so 3 layers x many windows don't exhaust
            # the register file.
            reg_cache = {}
            qn = [0]

            def nreg(v):
                if v not in reg_cache:
                    r = nc.gpsimd.alloc_register(f"nidx{v}")
                    nc.gpsimd.reg_mov(r, v)
                    reg_cache[v] = r
                return reg_cache[v]

            iota_sb = stp.tile([TILE, TILE], f16)
            nc.sync.dma_start(out=iota_sb[:], in_=iota_d[:])
            ident_sb = stp.tile([TILE, TILE], f16)
            nc.sync.dma_start(out=ident_sb[:], in_=ident_d[:])
            ident32_sb = stp.tile([TILE, TILE], f32)
            nc.vector.tensor_copy(ident32_sb[:], ident_sb[:])
            waug_sb = []
            for l, do in enumerate([D_F, D_F, d_out]):
                w = stp.tile([D_F + 1, do], f16, tag=f"waug{l}")
                nc.sync.dma_start(out=w[:], in_=waug_d[l][:])
                waug_sb.append(w)
            isd2_sb = stp.tile([TILE, ntiles], f32)
            nc.sync.dma_start(out=isd2_sb[:], in_=isd2_d[:])
            isdrow_sb = stp.tile([1, OWN], f32)
            nc.sync.dma_start(out=isdrow_sb[:], in_=isdrow_d[:])
            idxA_sb = stp.tile([128, SA // 16], i16)
            nc.sync.dma_start(out=idxA_sb[:], in_=idxA_d[:])
            idxB_sb = stp.tile([128, SB // 16], i16)
            nc.sync.dma_start(out=idxB_sb[:], in_=idxB_d[:])
            dstl_sb = stp.tile([128, nchunk], f16)
            nc.sync.dma_start(out=dstl_sb[:], in_=dstl_d[:])
            out_sb = stp.tile([1, OWN], f32)

            def emit_windows(st, idx_sb, gtab, nrows):
                # Each dma_gather covers WIN chunks (the SWDGE ring holds
                # ~1024 descs); windows round-robin the NQ queues so ring
                # drain overlaps desc-gen.
                gslab = gtab[0:nrows, :]
                nch_st = (SA if st == "A" else SB) // TILE
                lst = []
                for w in range(0, nch_st, WIN):
                    kw = min(WIN, nch_st - w)
                    wt = mp.tile([128, WIN * TILE], f16, tag=f"win{st}")
                    nc.gpsimd.dma_gather(
                        wt[:, : kw * TILE].rearrange("p (c e) -> p c e", e=TILE),
                        gslab,
                        idx_sb[:, w * 8 : (w + kw) * 8],
                        kw * TILE,
                        nreg(kw * TILE),
                        TILE,
                        queue_num=qn[0],
                    )
                    qn[0] = (qn[0] + 1) % NQ
                    lst.append(wt)
                return lst

            nlayers = 3 if stage >= 7 else 1  # stage 8: 3 layers, no CC
            if stage < 7:
                nc.vector.memset(out_sb[:], 0.0)
            wins = {
                "A": emit_windows("A", idxA_sb, g0A_d, NA),
                "B": emit_windows("B", idxB_sb, g0B_d, NB),
            }
            for layer in range(nlayers):
                gownA_src = [g0ownA_d, gownA_d[0], gownA_d[1]][layer]
                gownB_src = [g0ownB_d, gownB_d[0], gownB_d[1]][layer]
                do = D_F if layer < 2 else d_out

                def msg_lhs(st, chunk):
                    wt = wins[st][chunk // WIN]
                    col = (chunk % WIN) * TILE
                    return wt[:, col : col + D_F]

                for g, ts in enumerate(grp_tiles):
                    t0, t1 = ts[0], ts[-1] + 1
                    gw = (t1 - t0) * TILE
                    row0 = t0 * TILE
                    rows = min(gw, OWN - row0)
                    c0 = int(chunk_base[t0])
                    nch = int(chunk_base[t1] - c0)

                    # ---- one-hot scatter matrices for every chunk ----------
                    if stage < 2:
                        continue
                    S = sp.tile([128, max_ch * TILE], f16, tag="S")
                    nc.vector.tensor_tensor(
                        S[:, : nch * TILE].rearrange("p (c e) -> p c e", e=TILE),
                        iota_sb[:].unsqueeze(1).broadcast_to([TILE, nch, TILE]),
                        dstl_sb[:, c0 : c0 + nch]
                        .unsqueeze(2)
                        .broadcast_to([TILE, nch, TILE]),
                        mybir.AluOpType.is_equal,
                    )

                    # ---- own-node g rows for the self-loop term ------------
                    if stage < 3:
                        continue
                    gself = gp.tile([TILE, (t1 - t0) * D_F], f16, tag="gself")
                    if rows < gw:
                        nc.vector.memset(gself[:], 0.0)
                    for ti, t in enumerate(ts):
                        r0 = row0 + ti * TILE
                        r = min(TILE, OWN - r0)
                        if r0 < LO:
                            own_src = gownA_src[r0 : r0 + r, 0:D_F]
                        else:
                            own_src = gownB_src[r0 - LO : r0 - LO + r, 0:D_F]
                        nc.sync.dma_start(
                            out=gself[0:r, ti * D_F : ti * D_F + D_F],
                            in_=own_src,
                        )

                    # ---- seg-sum into PSUM, one region per dst tile --------
                    ps = p_ps.tile([D_F, gw], f32, space="PSUM", tag="ps")
                    for ti, t in enumerate(ts):
                        sl = slice(ti * TILE, (ti + 1) * TILE)
                        nmm = int(CA[t] + CB[t])
                        nc.tensor.matmul(
                            out=ps[:, sl],
                            lhsT=gself[:, ti * D_F : ti * D_F + D_F],
                            rhs=ident_sb[:],
                            start=True,
                            stop=(nmm == 0),
                        )
                        for j in range(nmm):
                            if j < CA[t]:
                                lhs = msg_lhs("A", int(a_off[t]) + j)
                            else:
                                jb = j - int(CA[t])
                                lhs = msg_lhs("B", int(b_off[t]) + jb)
                            scol = (int(chunk_base[t]) - c0 + j) * TILE
                            nc.tensor.matmul(
                                out=ps[:, sl],
                                lhsT=lhs,
                                rhs=S[:, scol : scol + TILE],
                                start=False,
                                stop=(j == nmm - 1),
                            )

                    # ---- augmented dense layer: Q = Waug^T @ [P; sigma] ----
                    if stage < 4:
                        continue
                    paug = pp.tile([D_F + 1, gw], f16, tag="paug")
                    nc.vector.tensor_copy(paug[0:D_F, :gw], ps[:, :gw])
                    nc.sync.dma_start(
                        out=paug[D_F : D_F + 1, 0:rows],
                        in_=sigma_d[:, row0 : row0 + rows],
                    )
                    if rows < gw:
                        nc.vector.memset(paug[D_F : D_F + 1, rows:gw], 0.0)
                    qs = q_ps.tile([D_F, gw], f32, space="PSUM", tag="qs")
                    nc.tensor.matmul(
                        out=qs[0:do, :gw],
                        lhsT=waug_sb[layer][:],
                        rhs=paug[:, :gw],
                        start=True,
                        stop=True,
                    )

                    if stage < 5:
                        continue
                    if layer < 2:
                        # g' = isd^2 * relu(Q), transposed back to node-major
                        qr = qp.tile([D_F, gw], f32, tag="qr")
                        nc.scalar.activation(
                            qr[:, :gw],
                            qs[0:D_F, :gw],
                            mybir.ActivationFunctionType.Relu,
                        )
                        for ti, t in enumerate(ts):
                            qt = t_ps.tile([TILE, D_F], f32, space="PSUM", tag="qt")
                            nc.tensor.transpose(
                                out=qt[:],
                                in_=qr[:, ti * TILE : (ti + 1) * TILE],
                                identity=ident32_sb[0:D_F, 0:D_F],
                            )
                            gsl = gop.tile([TILE, D_PAD], f16, tag="gsl")
                            nc.vector.memset(gsl[:, D_F:D_PAD], 0.0)
                            nc.vector.tensor_scalar_mul(
                                gsl[:, 0:D_F], qt[:], isd2_sb[:, t : t + 1]
                            )
                            r0 = row0 + ti * TILE
                            r = min(TILE, OWN - r0)
                            if r0 < LO:
                                own_dst = gownA_d[layer][r0 : r0 + r, :]
                            else:
                                own_dst = gownB_d[layer][r0 - LO : r0 - LO + r, :]
                            nc.sync.dma_start(out=own_dst, in_=gsl[0:r, :])
                    else:
                        nc.vector.tensor_copy(
                            out_sb[:, row0 : row0 + rows], qs[0:1, 0:rows]
                        )

                if layer < 2 and stage >= 6 and stage != 8:
                    # A-half collective lands while the B-half tiles still
                    # compute; the next layer's A gathers are emitted between
                    # the two collectives so they overlap the B transfer.
                    nc.gpsimd.collective_compute(
                        "AllGather",
                        mybir.AluOpType.bypass,
                        replica_groups=rg,
                        ins=[gownA_d[layer][:]],
                        outs=[gfullA_d[layer][:]],
                    )
                    winsA = emit_windows("A", idxA_sb, gfullA_d[layer], NA)
                    nc.gpsimd.collective_compute(
                        "AllGather",
                        mybir.AluOpType.bypass,
                        replica_groups=rg,
                        ins=[gownB_d[layer][:]],
                        outs=[gfullB_d[layer][:]],
                    )
                    wins = {
                        "A": winsA,
                        "B": emit_windows("B", idxB_sb, gfullB_d[layer], NB),
                    }

            # out = isd * Q2  (host reshapes [1, OWN] -> [OWN, 1])
            nc.vector.tensor_tensor(
                out_sb[:], out_sb[:], isdrow_sb[:], mybir.AluOpType.mult
            )
            nc.sync.dma_start(out=out_d[:], in_=out_sb[:])

    nc.compile()
    return nc


# ----------------------------------------------------------------------------
# entry point
# ----------------------------------------------------------------------------


def kernel(x, edge_index, W0, b0, W1, b1, W2, b2):
    from concourse.bass_utils import run_bass_kernel_spmd

    meta, in_maps = _prepare(x, edge_index, W0, b0, W1, b1, W2, b2)
    nc = _build(meta)
    res = run_bass_kernel_spmd(nc, in_maps, list(range(NC_CORES)))
    out = np.concatenate(
        [res.results[c]["out"].reshape(-1, 1) for c in range(NC_CORES)], axis=0
    )
    return out.astype(np.float32)


# revision 8
# speedup vs baseline: 1.0986x; 1.0986x over previous
"""3-layer GCN (GCNConvNet) on 8 Trainium2 NeuronCores.

Math refactor: with isd = 1/sqrt(deg+1) and self-loop edges folded in,
each GCN layer  h' = relu( D^-1/2 (A+I) D^-1/2 (h W^T + 1 b^T) )  becomes

    g      = isd**2 * relu(Q_prev)          (node-major "source features")
    P[n]   = sum_{e: dst(e)=n} g[src(e)]    (+ g[n] self term)
    Q[n]   = Waug^T @ [P[n]; sigma[n]]      (Waug = [W^T; b], sigma = row sums)
    h'     = relu(isd * Q) = isd * relu(Q)  -> g' = isd^2 * relu(Q)

so every per-edge coefficient disappears into per-node scaling and the
scatter matrices are pure one-hot.  The final layer output is isd * Q2.

Sharding: nodes split into 8 contiguous dst ranges (6250 each).  Each core
computes P for its own range over ALL edges.  Edge gathers use
nc.gpsimd.dma_gather (int16 indices) with 256B elements that each cover
TWO unpadded 64-feature fp16 rows; edges are bucketed by source-row parity
so each 128-edge chunk reads one 64-column half of its gathered window.

The source table is split in two permuted half-tables so that (a) element
indices stay far below 2^15 and (b) the inter-layer AllGather splits into
two independent halves:

  A: rows (src % 6250) <  3200 -> table row core(src)*3200 + local
  B: rows (src % 6250) >= 3200 -> table row core(src)*3050 + local-3200

After each layer the core's own A-half rows (dst tiles 0-24) finish first,
so the A AllGather is issued while tiles 25-48 still compute, and the next
layer's A-stream gathers start as soon as it lands -- overlapping the B
AllGather with real work on every engine.

The per-dst-tile chunk structure is derived from the actual edge data at
kernel() call time and padded to the max over the 8 cores so that all
cores run one shared NEFF (SPMD).
"""

import math
import numpy as np

NC_CORES = 8
TILE = 128
GRP_TILES = 4  # dst tiles fused per PSUM/matmul group (4*128 = 512 <= max N)
D_F = 64  # feature width of hidden layers
LO_TILES = 25  # dst tiles in the A (low) half of each core's own range
WIN = 8  # chunks per gather window (8*128 = 1024 descs = SWDGE ring limit)
NQ = 4  # SWDGE queues


# ----------------------------------------------------------------------------
# host-side graph preprocessing
# ----------------------------------------------------------------------------


def _wrap16(v):
    """[S] int -> [128, S//16] int16, index i at [i%16, i//16], replicated x8."""
    S = v.shape[0]
    assert S % 16 == 0
    w = v.reshape(S // 16, 16).T.astype(np.int16)
    return np.ascontiguousarray(np.tile(w, (8, 1)))


def _prepare(x, edge_index, W0, b0, W1, b1, W2, b2):
    x = np.asarray(x, dtype=np.float32)
    ei = np.asarray(edge_index)
    W0 = np.asarray(W0, np.float32)
    b0 = np.asarray(b0, np.float32)
    W1 = np.asarray(W1, np.float32)
    b1 = np.asarray(b1, np.float32)
    W2 = np.asarray(W2, np.float32)
    b2 = np.asarray(b2, np.float32)

    N = x.shape[0]
    assert N % NC_CORES == 0
    OWN = N // NC_CORES
    ntiles = (OWN + TILE - 1) // TILE
    LO = LO_TILES * TILE  # own-local rows in the A half
    HI = OWN - LO
    assert LO % 2 == 0 and HI % 2 == 0
    NA, NB = NC_CORES * LO, NC_CORES * HI
    assert max(NA, NB) // 2 <= 32768, "int16 element indices"
    src = ei[0].astype(np.int64)
    dst = ei[1].astype(np.int64)

    deg = np.bincount(dst, minlength=N).astype(np.float32) + 1.0
    isd = (1.0 / np.sqrt(deg)).astype(np.float32)
    sigma = (
        np.bincount(dst, weights=isd[src].astype(np.float64), minlength=N).astype(
            np.float32
        )
        + isd
    )

    g0 = (isd[:, None] * x).astype(np.float16)  # [N, 64] unpadded
    # permuted half tables: A row = core*LO + local (local < LO)
    #                      B row = core*HI + local - LO
    loc_all = np.arange(N) % OWN
    g0A = np.ascontiguousarray(g0[loc_all < LO])  # ordered by (core, local)
    g0B = np.ascontiguousarray(g0[loc_all >= LO])

    # ---- edge bucketing: (core, tile, half, parity) -------------------------
    s_core = src // OWN
    s_loc = src % OWN
    half = (s_loc >= LO).astype(np.int64)
    rowA = s_core * LO + s_loc  # valid where half==0
    rowB = s_core * HI + (s_loc - LO)  # valid where half==1
    row = np.where(half == 0, rowA, rowB)
    par = row % 2
    elem = row // 2
    core = dst // OWN
    tl = (dst % OWN) // TILE
    key = (((core * ntiles + tl) * 2 + half) * 2) + par
    order = np.argsort(key, kind="stable")
    s_elem = elem[order]
    s_dstl = (dst % OWN) % TILE
    s_dstl = s_dstl[order]
    counts = np.bincount(key, minlength=NC_CORES * ntiles * 4).reshape(
        NC_CORES, ntiles, 2, 2
    )
    starts = np.zeros(NC_CORES * ntiles * 4 + 1, np.int64)
    np.cumsum(counts.reshape(-1), out=starts[1:])

    # chunks per (tile, half, parity), shared across cores
    C4 = np.maximum(1, -(-counts.max(axis=0) // TILE)).astype(np.int64)
    # (>=1 keeps gather groups non-empty; pure-pad chunks are cheap)
    nA_t = C4[:, 0, 0] + C4[:, 0, 1]
    nB_t = C4[:, 1, 0] + C4[:, 1, 1]

    a_off = np.zeros(ntiles + 1, np.int64)  # chunk offsets into the A stream
    np.cumsum(nA_t, out=a_off[1:])
    b_off = np.zeros(ntiles + 1, np.int64)
    np.cumsum(nB_t, out=b_off[1:])
    chunk_base = np.zeros(ntiles + 1, np.int64)
    np.cumsum(nA_t + nB_t, out=chunk_base[1:])
    SA, SB = int(a_off[-1]) * TILE, int(b_off[-1]) * TILE
    nchunk = int(chunk_base[-1])

    per_core = []
    for c in range(NC_CORES):
        # pad slots must gather *something*; spread them over distinct
        # elements so they don't hammer one HBM line (S-col is -1 so the
        # gathered values never contribute).
        sA = np.arange(SA, dtype=np.int64) % (NA // 2)
        sB = np.arange(SB, dtype=np.int64) % (NB // 2)
        dstl_flat = np.full(nchunk * TILE, -1.0, np.float32)
        for t in range(ntiles):
            # class order within a tile: A0 | A1 | B0 | B1
            a_slot = a_off[t] * TILE
            d_slot = chunk_base[t] * TILE
            b_slot = b_off[t] * TILE
            for h in range(2):
                for p in range(2):
                    k = ((c * ntiles + t) * 2 + h) * 2 + p
                    lo, hi = starts[k], starts[k + 1]
                    n = hi - lo
                    if h == 0:
                        sA[a_slot : a_slot + n] = s_elem[lo:hi]
                        a_slot += C4[t, 0, p] * TILE
                    else:
                        sB[b_slot : b_slot + n] = s_elem[lo:hi]
                        b_slot += C4[t, 1, p] * TILE
                    dstl_flat[d_slot : d_slot + n] = s_dstl[lo:hi]
                    d_slot += C4[t, h, p] * TILE
        own = isd[c * OWN : (c + 1) * OWN] ** 2
        tmp = np.zeros(ntiles * TILE, np.float32)
        tmp[:OWN] = own
        isd2 = np.ascontiguousarray(tmp.reshape(ntiles, TILE).T)
        per_core.append(
            dict(
                idxA=_wrap16(sA),
                idxB=_wrap16(sB),
                dstl=np.ascontiguousarray(
                    dstl_flat.reshape(nchunk, TILE).T.astype(np.float16)
                ),
                sigma=sigma[c * OWN : (c + 1) * OWN]
                .astype(np.float16)
                .reshape(1, OWN),
                isd2=isd2,
                isdrow=isd[c * OWN : (c + 1) * OWN]
                .astype(np.float32)
                .reshape(1, OWN),
                g0ownA=np.ascontiguousarray(g0[c * OWN : c * OWN + LO]),
                g0ownB=np.ascontiguousarray(g0[c * OWN + LO : (c + 1) * OWN]),
            )
        )

    waug = []
    for W, b in ((W0, b0), (W1, b1), (W2, b2)):
        wa = np.zeros((D_F + 1, W.shape[0]), np.float16)
        wa[:D_F, :] = W.T.astype(np.float16)
        wa[D_F, :] = b.astype(np.float16)
        waug.append(wa)

    iota = np.tile(np.arange(TILE, dtype=np.float16), (TILE, 1))
    ident = np.eye(TILE, dtype=np.float16)

    meta = dict(
        N=N,
        OWN=OWN,
        LO=LO,
        HI=HI,
        NA=NA,
        NB=NB,
        ntiles=ntiles,
        C4=C4,
        a_off=a_off,
        b_off=b_off,
        chunk_base=chunk_base,
        SA=SA,
        SB=SB,
        nchunk=nchunk,
        d_out=W2.shape[0],
    )

    in_maps = []
    for c in range(NC_CORES):
        m = dict(per_core[c])
        m["g0A"] = g0A
        m["g0B"] = g0B
        m["waug0"] = waug[0]
        m["waug1"] = waug[1]
        m["waug2"] = waug[2]
        m["iota"] = iota
        m["ident"] = ident
        in_maps.append(m)
    return meta, in_maps


# ----------------------------------------------------------------------------
# device kernel
# ----------------------------------------------------------------------------


def _build(meta, stage=99, n_dev=NC_CORES):
    # stage gates for HW bisection: 1 gathers, 2 +S build, 3 +seg matmuls,
    # 4 +aug matmul, 5 +postproc/gown, 6 +collective, >=7 all three layers.
    import concourse.bacc as bacc
    import concourse.mybir as mybir
    from concourse.tile import TileContext

    f16 = mybir.dt.float16
    f32 = mybir.dt.float32
    i16 = mybir.dt.int16

    N = meta["N"]
    OWN = meta["OWN"]
    LO, HI = meta["LO"], meta["HI"]
    NA, NB = meta["NA"], meta["NB"]
    ntiles = meta["ntiles"]
    C4 = meta["C4"]
    a_off, b_off = meta["a_off"], meta["b_off"]
    chunk_base = meta["chunk_base"]
    SA, SB, nchunk = meta["SA"], meta["SB"], meta["nchunk"]
    d_out = meta["d_out"]

    ngrp = (ntiles + GRP_TILES - 1) // GRP_TILES
    grp_tiles = [
        list(range(g * GRP_TILES, min((g + 1) * GRP_TILES, ntiles)))
        for g in range(ngrp)
    ]
    max_ch = max(
        int(chunk_base[ts[-1] + 1] - chunk_base[ts[0]]) for ts in grp_tiles
    )

    nc = bacc.Bacc("TRN2", target_bir_lowering=False, num_devices=n_dev,
                  num_swdge_queues=NQ)

    g0A_d = nc.dram_tensor("g0A", [NA, D_F], f16, kind="ExternalInput")
    g0B_d = nc.dram_tensor("g0B", [NB, D_F], f16, kind="ExternalInput")
    g0ownA_d = nc.dram_tensor("g0ownA", [LO, D_F], f16, kind="ExternalInput")
    g0ownB_d = nc.dram_tensor("g0ownB", [HI, D_F], f16, kind="ExternalInput")
    idxA_d = nc.dram_tensor("idxA", [128, SA // 16], i16, kind="ExternalInput")
    idxB_d = nc.dram_tensor("idxB", [128, SB // 16], i16, kind="ExternalInput")
    dstl_d = nc.dram_tensor("dstl", [128, nchunk], f16, kind="ExternalInput")
    waug_d = [
        nc.dram_tensor(f"waug{l}", [D_F + 1, do], f16, kind="ExternalInput")
        for l, do in enumerate([D_F, D_F, d_out])
    ]
    sigma_d = nc.dram_tensor("sigma", [1, OWN], f16, kind="ExternalInput")
    isd2_d = nc.dram_tensor("isd2", [TILE, ntiles], f32, kind="ExternalInput")
    isdrow_d = nc.dram_tensor("isdrow", [1, OWN], f32, kind="ExternalInput")
    iota_d = nc.dram_tensor("iota", [TILE, TILE], f16, kind="ExternalInput")
    ident_d = nc.dram_tensor("ident", [TILE, TILE], f16, kind="ExternalInput")
    out_d = nc.dram_tensor("out", [1, OWN], f32, kind="ExternalOutput")

    gownA_d = [nc.dram_tensor(f"gownA{l}", [LO, D_F], f16) for l in (1, 2)]
    gownB_d = [nc.dram_tensor(f"gownB{l}", [HI, D_F], f16) for l in (1, 2)]
    gfullA_d = [
        nc.dram_tensor(f"gfullA{l}", [NA, D_F], f16, addr_space="Shared")
        for l in (1, 2)
    ]
    gfullB_d = [
        nc.dram_tensor(f"gfullB{l}", [NB, D_F], f16, addr_space="Shared")
        for l in (1, 2)
    ]

    rg = [list(range(NC_CORES))]

    with TileContext(nc) as tc:
        with (
            tc.tile_pool(name="static", bufs=1) as stp,
            tc.tile_pool(name="msgs", bufs=2 * NQ) as mp,
            tc.tile_pool(name="smat", bufs=2) as sp,
            tc.tile_pool(name="gself", bufs=2) as gp,
            tc.tile_pool(name="paug", bufs=2) as pp,
            tc.tile_pool(name="qrelu", bufs=2) as qp,
            tc.tile_pool(name="gout", bufs=3) as gop,
            tc.tile_pool(name="pps", bufs=2, space="PSUM") as p_ps,
            tc.tile_pool(name="qps", bufs=2, space="PSUM") as q_ps,
            tc.tile_pool(name="tps", bufs=2, space="PSUM") as t_ps,
        ):
            # dma_gather burns one GPSIMD register per distinct num_idxs via
            # to_reg; cache by value so 3 layers x many windows don't exhaust
            # the register file.
            reg_cache = {}
            qn = [0]

            def nreg(v):
                if v not in reg_cache:
                    r = nc.gpsimd.alloc_register(f"nidx{v}")
                    nc.gpsimd.reg_mov(r, v)
                    reg_cache[v] = r
                return reg_cache[v]

            iota_sb = stp.tile([TILE, TILE], f16)
            nc.sync.dma_start(out=iota_sb[:], in_=iota_d[:])
            ident_sb = stp.tile([TILE, TILE], f16)
            nc.sync.dma_start(out=ident_sb[:], in_=ident_d[:])
            ident32_sb = stp.tile([TILE, TILE], f32)
            nc.vector.tensor_copy(ident32_sb[:], ident_sb[:])
            waug_sb = []
            for l, do in enumerate([D_F, D_F, d_out]):
                w = stp.tile([D_F + 1, do], f16, tag=f"waug{l}")
                nc.sync.dma_start(out=w[:], in_=waug_d[l][:])
                waug_sb.append(w)
            isd2_sb = stp.tile([TILE, ntiles], f32)
            nc.sync.dma_start(out=isd2_sb[:], in_=isd2_d[:])
            isdrow_sb = stp.tile([1, OWN], f32)
            nc.sync.dma_start(out=isdrow_sb[:], in_=isdrow_d[:])
            idxA_sb = stp.tile([128, SA // 16], i16)
            nc.sync.dma_start(out=idxA_sb[:], in_=idxA_d[:])
            idxB_sb = stp.tile([128, SB // 16], i16)
            nc.sync.dma_start(out=idxB_sb[:], in_=idxB_d[:])
            dstl_sb = stp.tile([128, nchunk], f16)
            nc.sync.dma_start(out=dstl_sb[:], in_=dstl_d[:])
            out_sb = stp.tile([1, OWN], f32)

            def emit_windows(st, idx_sb, gtab, nrows):
                # Each dma_gather covers WIN chunks (the SWDGE ring holds
                # ~1024 descs); windows round-robin the NQ queues so ring
                # drain overlaps desc-gen.  Elements are 256B = 2 rows.
                gslab = gtab[0:nrows, :].rearrange("(a b) f -> a (b f)", b=2)
                nch_st = (SA if st == "A" else SB) // TILE
                lst = []
                for w in range(0, nch_st, WIN):
                    kw = min(WIN, nch_st - w)
                    wt = mp.tile([128, WIN * TILE], f16, tag=f"win{st}")
                    nc.gpsimd.dma_gather(
                        wt[:, : kw * TILE].rearrange("p (c e) -> p c e", e=TILE),
                        gslab,
                        idx_sb[:, w * 8 : (w + kw) * 8],
                        kw * TILE,
                        nreg(kw * TILE),
                        TILE,
                        queue_num=qn[0],
                    )
                    qn[0] = (qn[0] + 1) % NQ
                    lst.append(wt)
                return lst

            nlayers = 3 if stage >= 7 else 1  # stage 8: 3 layers, no CC
            if stage < 7:
                nc.vector.memset(out_sb[:], 0.0)
            wins = {
                "A": emit_windows("A", idxA_sb, g0A_d, NA),
                "B": emit_windows("B", idxB_sb, g0B_d, NB),
            }
            for layer in range(nlayers):
                gownA_src = [g0ownA_d, gownA_d[0], gownA_d[1]][layer]
                gownB_src = [g0ownB_d, gownB_d[0], gownB_d[1]][layer]
                do = D_F if layer < 2 else d_out

                def msg_lhs(st, chunk, parity):
                    wt = wins[st][chunk // WIN]
                    col = (chunk % WIN) * TILE + parity * D_F
                    return wt[:, col : col + D_F]

                for g, ts in enumerate(grp_tiles):
                    t0, t1 = ts[0], ts[-1] + 1
                    gw = (t1 - t0) * TILE
                    row0 = t0 * TILE
                    rows = min(gw, OWN - row0)
                    c0 = int(chunk_base[t0])
                    nch = int(chunk_base[t1] - c0)

                    # ---- one-hot scatter matrices for every chunk ----------
                    if stage < 2:
                        continue
                    S = sp.tile([128, max_ch * TILE], f16, tag="S")
                    nc.vector.tensor_tensor(
                        S[:, : nch * TILE].rearrange("p (c e) -> p c e", e=TILE),
                        iota_sb[:].unsqueeze(1).broadcast_to([TILE, nch, TILE]),
                        dstl_sb[:, c0 : c0 + nch]
                        .unsqueeze(2)
                        .broadcast_to([TILE, nch, TILE]),
                        mybir.AluOpType.is_equal,
                    )

                    # ---- own-node g rows for the self-loop term ------------
                    if stage < 3:
                        continue
                    gself = gp.tile([TILE, (t1 - t0) * D_F], f16, tag="gself")
                    if rows < gw:
                        nc.vector.memset(gself[:], 0.0)
                    for ti, t in enumerate(ts):
                        r0 = row0 + ti * TILE
                        r = min(TILE, OWN - r0)
                        if r0 < LO:
                            own_src = gownA_src[r0 : r0 + r, :]
                        else:
                            own_src = gownB_src[r0 - LO : r0 - LO + r, :]
                        nc.sync.dma_start(
                            out=gself[0:r, ti * D_F : ti * D_F + D_F],
                            in_=own_src,
                        )

                    # ---- seg-sum into PSUM, one region per dst tile --------
                    ps = p_ps.tile([D_F, gw], f32, space="PSUM", tag="ps")
                    for ti, t in enumerate(ts):
                        sl = slice(ti * TILE, (ti + 1) * TILE)
                        nA0, nA1 = int(C4[t, 0, 0]), int(C4[t, 0, 1])
                        nB0, nB1 = int(C4[t, 1, 0]), int(C4[t, 1, 1])
                        nmm = nA0 + nA1 + nB0 + nB1
                        nc.tensor.matmul(
                            out=ps[:, sl],
                            lhsT=gself[:, ti * D_F : ti * D_F + D_F],
                            rhs=ident_sb[:],
                            start=True,
                            stop=(nmm == 0),
                        )
                        for j in range(nmm):
                            if j < nA0 + nA1:
                                par = int(j >= nA0)
                                lhs = msg_lhs("A", int(a_off[t]) + j, par)
                            else:
                                jb = j - (nA0 + nA1)
                                par = int(jb >= nB0)
                                lhs = msg_lhs("B", int(b_off[t]) + jb, par)
                            scol = (int(chunk_base[t]) - c0 + j) * TILE
                            nc.tensor.matmul(
                                out=ps[:, sl],
                                lhsT=lhs,
                                rhs=S[:, scol : scol + TILE],
                                start=False,
                                stop=(j == nmm - 1),
                            )

                    # ---- augmented dense layer: Q = Waug^T @ [P; sigma] ----
                    if stage < 4:
                        continue
                    paug = pp.tile([D_F + 1, gw], f16, tag="paug")
                    nc.vector.tensor_copy(paug[0:D_F, :gw], ps[:, :gw])
                    nc.sync.dma_start(
                        out=paug[D_F : D_F + 1, 0:rows],
                        in_=sigma_d[:, row0 : row0 + rows],
                    )
                    if rows < gw:
                        nc.vector.memset(paug[D_F : D_F + 1, rows:gw], 0.0)
                    qs = q_ps.tile([D_F, gw], f32, space="PSUM", tag="qs")
                    nc.tensor.matmul(
                        out=qs[0:do, :gw],
                        lhsT=waug_sb[layer][:],
                        rhs=paug[:, :gw],
                        start=True,
                        stop=True,
                    )

                    if stage < 5:
                        continue
                    if layer < 2:
                        # g' = isd^2 * relu(Q), transposed back to node-major
                        qr = qp.tile([D_F, gw], f32, tag="qr")
                        nc.scalar.activation(
                            qr[:, :gw],
                            qs[0:D_F, :gw],
                            mybir.ActivationFunctionType.Relu,
                        )
                        for ti, t in enumerate(ts):
                            qt = t_ps.tile([TILE, D_F], f32, space="PSUM", tag="qt")
                            nc.tensor.transpose(
                                out=qt[:],
                                in_=qr[:, ti * TILE : (ti + 1) * TILE],
                                identity=ident32_sb[0:D_F, 0:D_F],
                            )
                            gsl = gop.tile([TILE, D_F], f16, tag="gsl")
                            nc.vector.tensor_scalar_mul(
                                gsl[:], qt[:], isd2_sb[:, t : t + 1]
                            )
                            r0 = row0 + ti * TILE
                            r = min(TILE, OWN - r0)
                            if r0 < LO:
                                own_dst = gownA_d[layer][r0 : r0 + r, :]
                            else:
                                own_dst = gownB_d[layer][r0 - LO : r0 - LO + r, :]
                            nc.sync.dma_start(out=own_dst, in_=gsl[0:r, :])
                    else:
                        nc.vector.tensor_copy(
                            out_sb[:, row0 : row0 + rows], qs[0:1, 0:rows]
                        )

                if layer < 2 and stage >= 6 and stage != 8:
                    # A-half collective lands while the B-half tiles still
                    # compute; the next layer's A gathers are emitted between
                    # the two collectives so they overlap the B transfer.
                    nc.gpsimd.collective_compute(
                        "AllGather",
                        mybir.AluOpType.bypass,
                        replica_groups=rg,
                        ins=[gownA_d[layer][:]],
                        outs=[gfullA_d[layer][:]],
                    )
                    winsA = emit_windows("A", idxA_sb, gfullA_d[layer], NA)
                    nc.gpsimd.collective_compute(
                        "AllGather",
                        mybir.AluOpType.bypass,
                        replica_groups=rg,
                        ins=[gownB_d[layer][:]],
                        outs=[gfullB_d[layer][:]],
                    )
                    wins = {
                        "A": winsA,
                        "B": emit_windows("B", idxB_sb, gfullB_d[layer], NB),
                    }

            # out = isd * Q2  (host reshapes [1, OWN] -> [OWN, 1])
            nc.vector.tensor_tensor(
                out_sb[:], out_sb[:], isdrow_sb[:], mybir.AluOpType.mult
            )
            nc.sync.dma_start(out=out_d[:], in_=out_sb[:])

    nc.compile()
    return nc


# ----------------------------------------------------------------------------
# entry point
# ----------------------------------------------------------------------------


def kernel(x, edge_index, W0, b0, W1, b1, W2, b2):
    from concourse.bass_utils import run_bass_kernel_spmd

    meta, in_maps = _prepare(x, edge_index, W0, b0, W1, b1, W2, b2)
    nc = _build(meta)
    res = run_bass_kernel_spmd(nc, in_maps, list(range(NC_CORES)))
    out = np.concatenate(
        [res.results[c]["out"].reshape(-1, 1) for c in range(NC_CORES)], axis=0
    )
    return out.astype(np.float32)


# revision 10
# speedup vs baseline: 1.2614x; 1.1482x over previous
"""3-layer GCN (GCNConvNet) on 8 Trainium2 NeuronCores.

Math refactor: with isd = 1/sqrt(deg+1) and self-loop edges folded in,
each GCN layer  h' = relu( D^-1/2 (A+I) D^-1/2 (h W^T + 1 b^T) )  becomes

    g      = isd**2 * relu(Q_prev)          (node-major "source features")
    P[n]   = sum_{e: dst(e)=n} g[src(e)]    (+ g[n] self term)
    Q[n]   = Waug^T @ [P[n]; sigma[n]]      (Waug = [W^T; b], sigma = row sums)
    h'     = relu(isd * Q) = isd * relu(Q)  -> g' = isd^2 * relu(Q)

so every per-edge coefficient disappears into per-node scaling and the
scatter matrices are pure one-hot.  The final layer output is isd * Q2.

Sharding: nodes split into 8 contiguous dst ranges (6250 each).  Each core
computes P for its own range over ALL edges.  Edge gathers use
nc.gpsimd.dma_gather (int16 indices) with 256B elements that each cover
TWO unpadded 64-feature fp16 rows; edges are bucketed by source-row parity
so each 128-edge chunk reads one 64-column half of its gathered window.

The source table is split in two permuted half-tables so that (a) element
indices stay far below 2^15 and (b) the inter-layer AllGather splits into
two independent halves:

  A: rows (src % 6250) <  3200 -> table row core(src)*3200 + local
  B: rows (src % 6250) >= 3200 -> table row core(src)*3050 + local-3200

After each layer the core's own A-half rows (dst tiles 0-24) finish first,
so the A AllGather is issued while tiles 25-48 still compute, and the next
layer's A-stream gathers start as soon as it lands -- overlapping the B
AllGather with real work on every engine.

The per-dst-tile chunk structure is derived from the actual edge data at
kernel() call time and padded to the max over the 8 cores so that all
cores run one shared NEFF (SPMD).
"""

import math
import numpy as np

NC_CORES = 8
TILE = 128
GRP_TILES = 4  # dst tiles fused per PSUM/matmul group (4*128 = 512 <= max N)
D_F = 64  # feature width of hidden layers
LO_TILES = 25  # dst tiles in the A (low) half of each core's own range
WIN = 8  # chunks per gather window (8*128 = 1024 descs = SWDGE ring limit)
NQ = 4  # SWDGE queues


# ----------------------------------------------------------------------------
# host-side graph preprocessing
# ----------------------------------------------------------------------------


def _wrap16(v):
    """[S] int -> [128, S//16] int16, index i at [i%16, i//16], replicated x8."""
    S = v.shape[0]
    assert S % 16 == 0
    w = v.reshape(S // 16, 16).T.astype(np.int16)
    return np.ascontiguousarray(np.tile(w, (8, 1)))


def _prepare(x, edge_index, W0, b0, W1, b1, W2, b2):
    x = np.asarray(x, dtype=np.float32)
    ei = np.asarray(edge_index)
    W0 = np.asarray(W0, np.float32)
    b0 = np.asarray(b0, np.float32)
    W1 = np.asarray(W1, np.float32)
    b1 = np.asarray(b1, np.float32)
    W2 = np.asarray(W2, np.float32)
    b2 = np.asarray(b2, np.float32)

    N = x.shape[0]
    assert N % NC_CORES == 0
    OWN = N // NC_CORES
    ntiles = (OWN + TILE - 1) // TILE
    LO = LO_TILES * TILE  # own-local rows in the A half
    HI = OWN - LO
    assert LO % 2 == 0 and HI % 2 == 0
    NA, NB = NC_CORES * LO, NC_CORES * HI
    assert max(NA, NB) // 2 <= 32768, "int16 element indices"
    src = ei[0].astype(np.int64)
    dst = ei[1].astype(np.int64)

    deg = np.bincount(dst, minlength=N).astype(np.float32) + 1.0
    isd = (1.0 / np.sqrt(deg)).astype(np.float32)
    sigma = (
        np.bincount(dst, weights=isd[src].astype(np.float64), minlength=N).astype(
            np.float32
        )
        + isd
    )

    g0 = (isd[:, None] * x).astype(np.float16)  # [N, 64] unpadded
    # permuted half tables: A row = core*LO + local (local < LO)
    #                      B row = core*HI + local - LO
    loc_all = np.arange(N) % OWN
    g0A = np.ascontiguousarray(g0[loc_all < LO])  # ordered by (core, local)
    g0B = np.ascontiguousarray(g0[loc_all >= LO])

    # ---- edge bucketing: (core, tile, half, parity) -------------------------
    s_core = src // OWN
    s_loc = src % OWN
    half = (s_loc >= LO).astype(np.int64)
    rowA = s_core * LO + s_loc  # valid where half==0
    rowB = s_core * HI + (s_loc - LO)  # valid where half==1
    row = np.where(half == 0, rowA, rowB)
    par = row % 2
    elem = row // 2
    core = dst // OWN
    tl = (dst % OWN) // TILE
    key = (((core * ntiles + tl) * 2 + half) * 2) + par
    order = np.argsort(key, kind="stable")
    s_elem = elem[order]
    s_dstl = (dst % OWN) % TILE
    s_dstl = s_dstl[order]
    counts = np.bincount(key, minlength=NC_CORES * ntiles * 4).reshape(
        NC_CORES, ntiles, 2, 2
    )
    starts = np.zeros(NC_CORES * ntiles * 4 + 1, np.int64)
    np.cumsum(counts.reshape(-1), out=starts[1:])

    # chunks per (tile, half, parity), shared across cores
    C4 = np.maximum(1, -(-counts.max(axis=0) // TILE)).astype(np.int64)
    # (>=1 keeps gather groups non-empty; pure-pad chunks are cheap)
    nA_t = C4[:, 0, 0] + C4[:, 0, 1]
    nB_t = C4[:, 1, 0] + C4[:, 1, 1]

    a_off = np.zeros(ntiles + 1, np.int64)  # chunk offsets into the A stream
    np.cumsum(nA_t, out=a_off[1:])
    b_off = np.zeros(ntiles + 1, np.int64)
    np.cumsum(nB_t, out=b_off[1:])
    chunk_base = np.zeros(ntiles + 1, np.int64)
    np.cumsum(nA_t + nB_t, out=chunk_base[1:])
    SA, SB = int(a_off[-1]) * TILE, int(b_off[-1]) * TILE
    nchunk = int(chunk_base[-1])

    per_core = []
    for c in range(NC_CORES):
        # pad slots must gather *something*; spread them over distinct
        # elements so they don't hammer one HBM line (S-col is -1 so the
        # gathered values never contribute).
        sA = np.arange(SA, dtype=np.int64) % (NA // 2)
        sB = np.arange(SB, dtype=np.int64) % (NB // 2)
        dstl_flat = np.full(nchunk * TILE, -1.0, np.float32)
        for t in range(ntiles):
            # class order within a tile: A0 | A1 | B0 | B1
            a_slot = a_off[t] * TILE
            d_slot = chunk_base[t] * TILE
            b_slot = b_off[t] * TILE
            for h in range(2):
                for p in range(2):
                    k = ((c * ntiles + t) * 2 + h) * 2 + p
                    lo, hi = starts[k], starts[k + 1]
                    n = hi - lo
                    if h == 0:
                        sA[a_slot : a_slot + n] = s_elem[lo:hi]
                        a_slot += C4[t, 0, p] * TILE
                    else:
                        sB[b_slot : b_slot + n] = s_elem[lo:hi]
                        b_slot += C4[t, 1, p] * TILE
                    dstl_flat[d_slot : d_slot + n] = s_dstl[lo:hi]
                    d_slot += C4[t, h, p] * TILE
        # layer-0 message windows are a pure permutation of host data:
        # precompute them so layer 0 needs no device-side gathers at all.
        m0A = g0A.reshape(NA // 2, 2 * D_F)[sA]
        m0B = g0B.reshape(NB // 2, 2 * D_F)[sB]
        m0A = np.ascontiguousarray(
            m0A.reshape(SA // TILE, TILE, 2 * D_F).transpose(1, 0, 2).reshape(
                TILE, SA
            )
        )
        m0B = np.ascontiguousarray(
            m0B.reshape(SB // TILE, TILE, 2 * D_F).transpose(1, 0, 2).reshape(
                TILE, SB
            )
        )
        own = isd[c * OWN : (c + 1) * OWN] ** 2
        tmp = np.zeros(ntiles * TILE, np.float32)
        tmp[:OWN] = own
        isd2 = np.ascontiguousarray(tmp.reshape(ntiles, TILE).T)
        per_core.append(
            dict(
                idxA=_wrap16(sA),
                idxB=_wrap16(sB),
                msgs0A=m0A,
                msgs0B=m0B,
                dstl=np.ascontiguousarray(
                    dstl_flat.reshape(nchunk, TILE).T.astype(np.float16)
                ),
                sigma=sigma[c * OWN : (c + 1) * OWN]
                .astype(np.float16)
                .reshape(1, OWN),
                isd2=isd2,
                isdrow=isd[c * OWN : (c + 1) * OWN]
                .astype(np.float32)
                .reshape(1, OWN),
                g0ownA=np.ascontiguousarray(g0[c * OWN : c * OWN + LO]),
                g0ownB=np.ascontiguousarray(g0[c * OWN + LO : (c + 1) * OWN]),
            )
        )

    waug = []
    for W, b in ((W0, b0), (W1, b1), (W2, b2)):
        wa = np.zeros((D_F + 1, W.shape[0]), np.float16)
        wa[:D_F, :] = W.T.astype(np.float16)
        wa[D_F, :] = b.astype(np.float16)
        waug.append(wa)

    iota = np.tile(np.arange(TILE, dtype=np.float16), (TILE, 1))
    ident = np.eye(TILE, dtype=np.float16)

    meta = dict(
        N=N,
        OWN=OWN,
        LO=LO,
        HI=HI,
        NA=NA,
        NB=NB,
        ntiles=ntiles,
        C4=C4,
        a_off=a_off,
        b_off=b_off,
        chunk_base=chunk_base,
        SA=SA,
        SB=SB,
        nchunk=nchunk,
        d_out=W2.shape[0],
    )

    in_maps = []
    for c in range(NC_CORES):
        m = dict(per_core[c])
        m["waug0"] = waug[0]
        m["waug1"] = waug[1]
        m["waug2"] = waug[2]
        m["iota"] = iota
        m["ident"] = ident
        in_maps.append(m)
    return meta, in_maps


# ----------------------------------------------------------------------------
# device kernel
# ----------------------------------------------------------------------------


def _build(meta, stage=99, n_dev=NC_CORES):
    # stage gates for HW bisection: 1 gathers, 2 +S build, 3 +seg matmuls,
    # 4 +aug matmul, 5 +postproc/gown, 6 +collective, >=7 all three layers.
    import concourse.bacc as bacc
    import concourse.mybir as mybir
    from concourse.tile import TileContext

    f16 = mybir.dt.float16
    f32 = mybir.dt.float32
    i16 = mybir.dt.int16

    N = meta["N"]
    OWN = meta["OWN"]
    LO, HI = meta["LO"], meta["HI"]
    NA, NB = meta["NA"], meta["NB"]
    ntiles = meta["ntiles"]
    C4 = meta["C4"]
    a_off, b_off = meta["a_off"], meta["b_off"]
    chunk_base = meta["chunk_base"]
    SA, SB, nchunk = meta["SA"], meta["SB"], meta["nchunk"]
    d_out = meta["d_out"]

    ngrp = (ntiles + GRP_TILES - 1) // GRP_TILES
    grp_tiles = [
        list(range(g * GRP_TILES, min((g + 1) * GRP_TILES, ntiles)))
        for g in range(ngrp)
    ]
    max_ch = max(
        int(chunk_base[ts[-1] + 1] - chunk_base[ts[0]]) for ts in grp_tiles
    )

    nc = bacc.Bacc("TRN2", target_bir_lowering=False, num_devices=n_dev,
                  num_swdge_queues=NQ)

    msgs0A_d = nc.dram_tensor("msgs0A", [128, SA], f16,
                              kind="ExternalInput")
    msgs0B_d = nc.dram_tensor("msgs0B", [128, SB], f16,
                              kind="ExternalInput")
    g0ownA_d = nc.dram_tensor("g0ownA", [LO, D_F], f16, kind="ExternalInput")
    g0ownB_d = nc.dram_tensor("g0ownB", [HI, D_F], f16, kind="ExternalInput")
    idxA_d = nc.dram_tensor("idxA", [128, SA // 16], i16, kind="ExternalInput")
    idxB_d = nc.dram_tensor("idxB", [128, SB // 16], i16, kind="ExternalInput")
    dstl_d = nc.dram_tensor("dstl", [128, nchunk], f16, kind="ExternalInput")
    waug_d = [
        nc.dram_tensor(f"waug{l}", [D_F + 1, do], f16, kind="ExternalInput")
        for l, do in enumerate([D_F, D_F, d_out])
    ]
    sigma_d = nc.dram_tensor("sigma", [1, OWN], f16, kind="ExternalInput")
    isd2_d = nc.dram_tensor("isd2", [TILE, ntiles], f32, kind="ExternalInput")
    isdrow_d = nc.dram_tensor("isdrow", [1, OWN], f32, kind="ExternalInput")
    iota_d = nc.dram_tensor("iota", [TILE, TILE], f16, kind="ExternalInput")
    ident_d = nc.dram_tensor("ident", [TILE, TILE], f16, kind="ExternalInput")
    out_d = nc.dram_tensor("out", [1, OWN], f32, kind="ExternalOutput")

    gownA_d = [nc.dram_tensor(f"gownA{l}", [LO, D_F], f16) for l in (1, 2)]
    gownB_d = [nc.dram_tensor(f"gownB{l}", [HI, D_F], f16) for l in (1, 2)]
    gfullA_d = [
        nc.dram_tensor(f"gfullA{l}", [NA, D_F], f16, addr_space="Shared")
        for l in (1, 2)
    ]
    gfullB_d = [
        nc.dram_tensor(f"gfullB{l}", [NB, D_F], f16, addr_space="Shared")
        for l in (1, 2)
    ]

    rg = [list(range(NC_CORES))]

    with TileContext(nc) as tc:
        with (
            tc.tile_pool(name="static", bufs=1) as stp,
            tc.tile_pool(name="msgs", bufs=2 * NQ) as mp,
            tc.tile_pool(name="smat", bufs=2) as sp,
            tc.tile_pool(name="gself", bufs=2) as gp,
            tc.tile_pool(name="paug", bufs=2) as pp,
            tc.tile_pool(name="qrelu", bufs=2) as qp,
            tc.tile_pool(name="gout", bufs=3) as gop,
            tc.tile_pool(name="pps", bufs=2, space="PSUM") as p_ps,
            tc.tile_pool(name="qps", bufs=2, space="PSUM") as q_ps,
            tc.tile_pool(name="tps", bufs=2, space="PSUM") as t_ps,
        ):
            # dma_gather burns one GPSIMD register per distinct num_idxs via
            # to_reg; cache by value so 3 layers x many windows don't exhaust
            # the register file.
            reg_cache = {}
            qn = [0]

            def nreg(v):
                if v not in reg_cache:
                    r = nc.gpsimd.alloc_register(f"nidx{v}")
                    nc.gpsimd.reg_mov(r, v)
                    reg_cache[v] = r
                return reg_cache[v]

            iota_sb = stp.tile([TILE, TILE], f16)
            nc.sync.dma_start(out=iota_sb[:], in_=iota_d[:])
            ident_sb = stp.tile([TILE, TILE], f16)
            nc.sync.dma_start(out=ident_sb[:], in_=ident_d[:])
            ident32_sb = stp.tile([TILE, TILE], f32)
            nc.vector.tensor_copy(ident32_sb[:], ident_sb[:])
            waug_sb = []
            for l, do in enumerate([D_F, D_F, d_out]):
                w = stp.tile([D_F + 1, do], f16, tag=f"waug{l}")
                nc.sync.dma_start(out=w[:], in_=waug_d[l][:])
                waug_sb.append(w)
            isd2_sb = stp.tile([TILE, ntiles], f32)
            nc.sync.dma_start(out=isd2_sb[:], in_=isd2_d[:])
            isdrow_sb = stp.tile([1, OWN], f32)
            nc.sync.dma_start(out=isdrow_sb[:], in_=isdrow_d[:])
            idxA_sb = stp.tile([128, SA // 16], i16)
            nc.sync.dma_start(out=idxA_sb[:], in_=idxA_d[:])
            idxB_sb = stp.tile([128, SB // 16], i16)
            nc.sync.dma_start(out=idxB_sb[:], in_=idxB_d[:])
            dstl_sb = stp.tile([128, nchunk], f16)
            nc.sync.dma_start(out=dstl_sb[:], in_=dstl_d[:])
            out_sb = stp.tile([1, OWN], f32)

            def emit_windows(st, idx_sb, gtab, nrows):
                # Each dma_gather covers WIN chunks (the SWDGE ring holds
                # ~1024 descs); windows round-robin the NQ queues so ring
                # drain overlaps desc-gen.  Elements are 256B = 2 rows.
                gslab = gtab[0:nrows, :].rearrange("(a b) f -> a (b f)", b=2)
                nch_st = (SA if st == "A" else SB) // TILE
                lst = []
                for w in range(0, nch_st, WIN):
                    kw = min(WIN, nch_st - w)
                    wt = mp.tile([128, WIN * TILE], f16, tag=f"win{st}")
                    nc.gpsimd.dma_gather(
                        wt[:, : kw * TILE].rearrange("p (c e) -> p c e", e=TILE),
                        gslab,
                        idx_sb[:, w * 8 : (w + kw) * 8],
                        kw * TILE,
                        nreg(kw * TILE),
                        TILE,
                        queue_num=qn[0],
                    )
                    qn[0] = (qn[0] + 1) % NQ
                    lst.append(wt)
                return lst

            def emit_windows_dram(st, src_d):
                # layer 0: windows are plain HWDGE loads of host-prepacked
                # messages -- zero GPSIMD descriptor generation.
                nch_st = (SA if st == "A" else SB) // TILE
                lst = []
                for w in range(0, nch_st, WIN):
                    kw = min(WIN, nch_st - w)
                    wt = mp.tile([128, WIN * TILE], f16, tag=f"win{st}")
                    nc.sync.dma_start(
                        out=wt[:, : kw * TILE],
                        in_=src_d[:, w * TILE : (w + kw) * TILE],
                    )
                    lst.append(wt)
                return lst

            nlayers = 3 if stage >= 7 else 1  # stage 8: 3 layers, no CC
            if stage < 7:
                nc.vector.memset(out_sb[:], 0.0)
            wins = {
                "A": emit_windows_dram("A", msgs0A_d),
                "B": emit_windows_dram("B", msgs0B_d),
            }
            for layer in range(nlayers):
                gownA_src = [g0ownA_d, gownA_d[0], gownA_d[1]][layer]
                gownB_src = [g0ownB_d, gownB_d[0], gownB_d[1]][layer]
                do = D_F if layer < 2 else d_out

                def msg_lhs(st, chunk, parity):
                    wt = wins[st][chunk // WIN]
                    col = (chunk % WIN) * TILE + parity * D_F
                    return wt[:, col : col + D_F]

                for g, ts in enumerate(grp_tiles):
                    t0, t1 = ts[0], ts[-1] + 1
                    gw = (t1 - t0) * TILE
                    row0 = t0 * TILE
                    rows = min(gw, OWN - row0)
                    c0 = int(chunk_base[t0])
                    nch = int(chunk_base[t1] - c0)

                    # ---- one-hot scatter matrices for every chunk ----------
                    if stage < 2:
                        continue
                    S = sp.tile([128, max_ch * TILE], f16, tag="S")
                    nc.vector.tensor_tensor(
                        S[:, : nch * TILE].rearrange("p (c e) -> p c e", e=TILE),
                        iota_sb[:].unsqueeze(1).broadcast_to([TILE, nch, TILE]),
                        dstl_sb[:, c0 : c0 + nch]
                        .unsqueeze(2)
                        .broadcast_to([TILE, nch, TILE]),
                        mybir.AluOpType.is_equal,
                    )

                    # ---- own-node g rows for the self-loop term ------------
                    if stage < 3:
                        continue
                    gself = gp.tile([TILE, (t1 - t0) * D_F], f16, tag="gself")
                    if rows < gw:
                        nc.vector.memset(gself[:], 0.0)
                    for ti, t in enumerate(ts):
                        r0 = row0 + ti * TILE
                        r = min(TILE, OWN - r0)
                        if r0 < LO:
                            own_src = gownA_src[r0 : r0 + r, :]
                        else:
                            own_src = gownB_src[r0 - LO : r0 - LO + r, :]
                        nc.sync.dma_start(
                            out=gself[0:r, ti * D_F : ti * D_F + D_F],
                            in_=own_src,
                        )

                    # ---- seg-sum into PSUM, one region per dst tile --------
                    ps = p_ps.tile([D_F, gw], f32, space="PSUM", tag="ps")
                    for ti, t in enumerate(ts):
                        sl = slice(ti * TILE, (ti + 1) * TILE)
                        nA0, nA1 = int(C4[t, 0, 0]), int(C4[t, 0, 1])
                        nB0, nB1 = int(C4[t, 1, 0]), int(C4[t, 1, 1])
                        nmm = nA0 + nA1 + nB0 + nB1
                        nc.tensor.matmul(
                            out=ps[:, sl],
                            lhsT=gself[:, ti * D_F : ti * D_F + D_F],
                            rhs=ident_sb[:],
                            start=True,
                            stop=(nmm == 0),
                        )
                        for j in range(nmm):
                            if j < nA0 + nA1:
                                par = int(j >= nA0)
                                lhs = msg_lhs("A", int(a_off[t]) + j, par)
                            else:
                                jb = j - (nA0 + nA1)
                                par = int(jb >= nB0)
                                lhs = msg_lhs("B", int(b_off[t]) + jb, par)
                            scol = (int(chunk_base[t]) - c0 + j) * TILE
                            nc.tensor.matmul(
                                out=ps[:, sl],
                                lhsT=lhs,
                                rhs=S[:, scol : scol + TILE],
                                start=False,
                                stop=(j == nmm - 1),
                            )

                    # ---- augmented dense layer: Q = Waug^T @ [P; sigma] ----
                    if stage < 4:
                        continue
                    paug = pp.tile([D_F + 1, gw], f16, tag="paug")
                    nc.vector.tensor_copy(paug[0:D_F, :gw], ps[:, :gw])
                    nc.sync.dma_start(
                        out=paug[D_F : D_F + 1, 0:rows],
                        in_=sigma_d[:, row0 : row0 + rows],
                    )
                    if rows < gw:
                        nc.vector.memset(paug[D_F : D_F + 1, rows:gw], 0.0)
                    qs = q_ps.tile([D_F, gw], f32, space="PSUM", tag="qs")
                    nc.tensor.matmul(
                        out=qs[0:do, :gw],
                        lhsT=waug_sb[layer][:],
                        rhs=paug[:, :gw],
                        start=True,
                        stop=True,
                    )

                    if stage < 5:
                        continue
                    if layer < 2:
                        # g' = isd^2 * relu(Q), transposed back to node-major
                        qr = qp.tile([D_F, gw], f32, tag="qr")
                        nc.scalar.activation(
                            qr[:, :gw],
                            qs[0:D_F, :gw],
                            mybir.ActivationFunctionType.Relu,
                        )
                        for ti, t in enumerate(ts):
                            qt = t_ps.tile([TILE, D_F], f32, space="PSUM", tag="qt")
                            nc.tensor.transpose(
                                out=qt[:],
                                in_=qr[:, ti * TILE : (ti + 1) * TILE],
                                identity=ident32_sb[0:D_F, 0:D_F],
                            )
                            gsl = gop.tile([TILE, D_F], f16, tag="gsl")
                            nc.vector.tensor_scalar_mul(
                                gsl[:], qt[:], isd2_sb[:, t : t + 1]
                            )
                            r0 = row0 + ti * TILE
                            r = min(TILE, OWN - r0)
                            if r0 < LO:
                                own_dst = gownA_d[layer][r0 : r0 + r, :]
                            else:
                                own_dst = gownB_d[layer][r0 - LO : r0 - LO + r, :]
                            nc.sync.dma_start(out=own_dst, in_=gsl[0:r, :])
                    else:
                        nc.vector.tensor_copy(
                            out_sb[:, row0 : row0 + rows], qs[0:1, 0:rows]
                        )

                if layer < 2 and stage >= 6 and stage != 8:
                    # A-half collective lands while the B-half tiles still
                    # compute; the next layer's A gathers are emitted between
                    # the two collectives so they overlap the B transfer.
                    nc.gpsimd.collective_compute(
                        "AllGather",
                        mybir.AluOpType.bypass,
                        replica_groups=rg,
                        ins=[gownA_d[layer][:]],
                        outs=[gfullA_d[layer][:]],
                    )
                    winsA = emit_windows("A", idxA_sb, gfullA_d[layer], NA)
                    nc.gpsimd.collective_compute(
                        "AllGather",
                        mybir.AluOpType.bypass,
                        replica_groups=rg,
                        ins=[gownB_d[layer][:]],
                        outs=[gfullB_d[layer][:]],
                    )
                    wins = {
                        "A": winsA,
                        "B": emit_windows("B", idxB_sb, gfullB_d[layer], NB),
                    }

            # out = isd * Q2  (host reshapes [1, OWN] -> [OWN, 1])
            nc.vector.tensor_tensor(
                out_sb[:], out_sb[:], isdrow_sb[:], mybir.AluOpType.mult
            )
            nc.sync.dma_start(out=out_d[:], in_=out_sb[:])

    nc.compile()
    return nc


# ----------------------------------------------------------------------------
# entry point
# ----------------------------------------------------------------------------


def kernel(x, edge_index, W0, b0, W1, b1, W2, b2):
    from concourse.bass_utils import run_bass_kernel_spmd

    meta, in_maps = _prepare(x, edge_index, W0, b0, W1, b1, W2, b2)
    nc = _build(meta)
    res = run_bass_kernel_spmd(nc, in_maps, list(range(NC_CORES)))
    out = np.concatenate(
        [res.results[c]["out"].reshape(-1, 1) for c in range(NC_CORES)], axis=0
    )
    return out.astype(np.float32)


# revision 11
# speedup vs baseline: 1.2714x; 1.0079x over previous
"""3-layer GCN (GCNConvNet) on 8 Trainium2 NeuronCores.

Math refactor: with isd = 1/sqrt(deg+1) and self-loop edges folded in,
each GCN layer  h' = relu( D^-1/2 (A+I) D^-1/2 (h W^T + 1 b^T) )  becomes

    g      = isd**2 * relu(Q_prev)          (node-major "source features")
    P[n]   = sum_{e: dst(e)=n} g[src(e)]    (+ g[n] self term)
    Q[n]   = Waug^T @ [P[n]; sigma[n]]      (Waug = [W^T; b], sigma = row sums)
    h'     = relu(isd * Q) = isd * relu(Q)  -> g' = isd^2 * relu(Q)

so every per-edge coefficient disappears into per-node scaling and the
scatter matrices are pure one-hot.  The final layer output is isd * Q2.

Sharding: nodes split into 8 contiguous dst ranges (6250 each).  Each core
computes P for its own range over ALL edges.  Edge gathers use
nc.gpsimd.dma_gather (int16 indices) with 256B elements that each cover
TWO unpadded 64-feature fp16 rows; edges are bucketed by source-row parity
so each 128-edge chunk reads one 64-column half of its gathered window.

The source table is split in two permuted half-tables so that (a) element
indices stay far below 2^15 and (b) the inter-layer AllGather splits into
two independent halves:

  A: rows (src % 6250) <  3200 -> table row core(src)*3200 + local
  B: rows (src % 6250) >= 3200 -> table row core(src)*3050 + local-3200

After each layer the core's own A-half rows (dst tiles 0-24) finish first,
so the A AllGather is issued while tiles 25-48 still compute, and the next
layer's A-stream gathers start as soon as it lands -- overlapping the B
AllGather with real work on every engine.

The per-dst-tile chunk structure is derived from the actual edge data at
kernel() call time and padded to the max over the 8 cores so that all
cores run one shared NEFF (SPMD).
"""

import math
import numpy as np

NC_CORES = 8
TILE = 128
GRP_TILES = 4  # dst tiles fused per PSUM/matmul group (4*128 = 512 <= max N)
D_F = 64  # feature width of hidden layers
LO_TILES = 25  # dst tiles in the A (low) half of each core's own range
WIN = 8  # chunks per gather window (8*128 = 1024 descs = SWDGE ring limit)
NQ = 4  # SWDGE queues


# ----------------------------------------------------------------------------
# host-side graph preprocessing
# ----------------------------------------------------------------------------


def _wrap16(v):
    """[S] int -> [128, S//16] int16, index i at [i%16, i//16], replicated x8."""
    S = v.shape[0]
    assert S % 16 == 0
    w = v.reshape(S // 16, 16).T.astype(np.int16)
    return np.ascontiguousarray(np.tile(w, (8, 1)))


def _prepare(x, edge_index, W0, b0, W1, b1, W2, b2):
    x = np.asarray(x, dtype=np.float32)
    ei = np.asarray(edge_index)
    W0 = np.asarray(W0, np.float32)
    b0 = np.asarray(b0, np.float32)
    W1 = np.asarray(W1, np.float32)
    b1 = np.asarray(b1, np.float32)
    W2 = np.asarray(W2, np.float32)
    b2 = np.asarray(b2, np.float32)

    N = x.shape[0]
    assert N % NC_CORES == 0
    OWN = N // NC_CORES
    ntiles = (OWN + TILE - 1) // TILE
    LO = LO_TILES * TILE  # own-local rows in the A half
    HI = OWN - LO
    assert LO % 2 == 0 and HI % 2 == 0
    NA, NB = NC_CORES * LO, NC_CORES * HI
    assert max(NA, NB) // 2 <= 32768, "int16 element indices"
    src = ei[0].astype(np.int64)
    dst = ei[1].astype(np.int64)

    deg = np.bincount(dst, minlength=N).astype(np.float32) + 1.0
    isd = (1.0 / np.sqrt(deg)).astype(np.float32)
    sigma = (
        np.bincount(dst, weights=isd[src].astype(np.float64), minlength=N).astype(
            np.float32
        )
        + isd
    )

    g0 = (isd[:, None] * x).astype(np.float16)  # [N, 64] unpadded
    # permuted half tables: A row = core*LO + local (local < LO)
    #                      B row = core*HI + local - LO
    loc_all = np.arange(N) % OWN
    g0A = np.ascontiguousarray(g0[loc_all < LO])  # ordered by (core, local)
    g0B = np.ascontiguousarray(g0[loc_all >= LO])

    # ---- edge bucketing: (core, tile, half, parity) -------------------------
    s_core = src // OWN
    s_loc = src % OWN
    half = (s_loc >= LO).astype(np.int64)
    rowA = s_core * LO + s_loc  # valid where half==0
    rowB = s_core * HI + (s_loc - LO)  # valid where half==1
    row = np.where(half == 0, rowA, rowB)
    par = row % 2
    elem = row // 2
    core = dst // OWN
    tl = (dst % OWN) // TILE
    key = (((core * ntiles + tl) * 2 + half) * 2) + par
    order = np.argsort(key, kind="stable")
    s_elem = elem[order]
    s_dstl = (dst % OWN) % TILE
    s_dstl = s_dstl[order]
    counts = np.bincount(key, minlength=NC_CORES * ntiles * 4).reshape(
        NC_CORES, ntiles, 2, 2
    )
    starts = np.zeros(NC_CORES * ntiles * 4 + 1, np.int64)
    np.cumsum(counts.reshape(-1), out=starts[1:])

    # chunks per (tile, half, parity), shared across cores
    C4 = np.maximum(1, -(-counts.max(axis=0) // TILE)).astype(np.int64)
    # (>=1 keeps gather groups non-empty; pure-pad chunks are cheap)
    nA_t = C4[:, 0, 0] + C4[:, 0, 1]
    nB_t = C4[:, 1, 0] + C4[:, 1, 1]

    a_off = np.zeros(ntiles + 1, np.int64)  # chunk offsets into the A stream
    np.cumsum(nA_t, out=a_off[1:])
    b_off = np.zeros(ntiles + 1, np.int64)
    np.cumsum(nB_t, out=b_off[1:])
    chunk_base = np.zeros(ntiles + 1, np.int64)
    np.cumsum(nA_t + nB_t, out=chunk_base[1:])
    SA, SB = int(a_off[-1]) * TILE, int(b_off[-1]) * TILE
    nchunk = int(chunk_base[-1])

    per_core = []
    for c in range(NC_CORES):
        # pad slots must gather *something*; spread them over distinct
        # elements so they don't hammer one HBM line (S-col is -1 so the
        # gathered values never contribute).
        sA = np.arange(SA, dtype=np.int64) % (NA // 2)
        sB = np.arange(SB, dtype=np.int64) % (NB // 2)
        dstl_flat = np.full(nchunk * TILE, -1.0, np.float32)
        for t in range(ntiles):
            # class order within a tile: A0 | A1 | B0 | B1
            a_slot = a_off[t] * TILE
            d_slot = chunk_base[t] * TILE
            b_slot = b_off[t] * TILE
            for h in range(2):
                for p in range(2):
                    k = ((c * ntiles + t) * 2 + h) * 2 + p
                    lo, hi = starts[k], starts[k + 1]
                    n = hi - lo
                    if h == 0:
                        sA[a_slot : a_slot + n] = s_elem[lo:hi]
                        a_slot += C4[t, 0, p] * TILE
                    else:
                        sB[b_slot : b_slot + n] = s_elem[lo:hi]
                        b_slot += C4[t, 1, p] * TILE
                    dstl_flat[d_slot : d_slot + n] = s_dstl[lo:hi]
                    d_slot += C4[t, h, p] * TILE
        # layer-0 message windows are a pure permutation of host data:
        # precompute them so layer 0 needs no device-side gathers at all.
        m0A = g0A.reshape(NA // 2, 2 * D_F)[sA]
        m0B = g0B.reshape(NB // 2, 2 * D_F)[sB]
        m0A = np.ascontiguousarray(
            m0A.reshape(SA // TILE, TILE, 2 * D_F).transpose(1, 0, 2).reshape(
                TILE, SA
            )
        )
        m0B = np.ascontiguousarray(
            m0B.reshape(SB // TILE, TILE, 2 * D_F).transpose(1, 0, 2).reshape(
                TILE, SB
            )
        )
        own = isd[c * OWN : (c + 1) * OWN] ** 2
        tmp = np.zeros(ntiles * TILE, np.float32)
        tmp[:OWN] = own
        isd2 = np.ascontiguousarray(tmp.reshape(ntiles, TILE).T)
        per_core.append(
            dict(
                idxA=_wrap16(sA),
                idxB=_wrap16(sB),
                msgs0A=m0A,
                msgs0B=m0B,
                dstl=np.ascontiguousarray(
                    dstl_flat.reshape(nchunk, TILE).T.astype(np.float16)
                ),
                sigma=sigma[c * OWN : (c + 1) * OWN]
                .astype(np.float16)
                .reshape(1, OWN),
                isd2=isd2,
                isdrow=isd[c * OWN : (c + 1) * OWN]
                .astype(np.float32)
                .reshape(1, OWN),
                g0ownA=np.ascontiguousarray(g0[c * OWN : c * OWN + LO]),
                g0ownB=np.ascontiguousarray(g0[c * OWN + LO : (c + 1) * OWN]),
            )
        )

    waug = []
    for W, b in ((W0, b0), (W1, b1), (W2, b2)):
        wa = np.zeros((D_F + 1, W.shape[0]), np.float16)
        wa[:D_F, :] = W.T.astype(np.float16)
        wa[D_F, :] = b.astype(np.float16)
        waug.append(wa)

    iota = np.tile(np.arange(TILE, dtype=np.float16), (TILE, 1))
    ident = np.eye(TILE, dtype=np.float16)

    meta = dict(
        N=N,
        OWN=OWN,
        LO=LO,
        HI=HI,
        NA=NA,
        NB=NB,
        ntiles=ntiles,
        C4=C4,
        a_off=a_off,
        b_off=b_off,
        chunk_base=chunk_base,
        SA=SA,
        SB=SB,
        nchunk=nchunk,
        d_out=W2.shape[0],
    )

    in_maps = []
    for c in range(NC_CORES):
        m = dict(per_core[c])
        m["waug0"] = waug[0]
        m["waug1"] = waug[1]
        m["waug2"] = waug[2]
        m["iota"] = iota
        m["ident"] = ident
        in_maps.append(m)
    return meta, in_maps


# ----------------------------------------------------------------------------
# device kernel
# ----------------------------------------------------------------------------


def _build(meta, stage=99, n_dev=NC_CORES):
    # stage gates for HW bisection: 1 gathers, 2 +S build, 3 +seg matmuls,
    # 4 +aug matmul, 5 +postproc/gown, 6 +collective, >=7 all three layers.
    import concourse.bacc as bacc
    import concourse.mybir as mybir
    from concourse.tile import TileContext

    f16 = mybir.dt.float16
    f32 = mybir.dt.float32
    i16 = mybir.dt.int16

    N = meta["N"]
    OWN = meta["OWN"]
    LO, HI = meta["LO"], meta["HI"]
    NA, NB = meta["NA"], meta["NB"]
    ntiles = meta["ntiles"]
    C4 = meta["C4"]
    a_off, b_off = meta["a_off"], meta["b_off"]
    chunk_base = meta["chunk_base"]
    SA, SB, nchunk = meta["SA"], meta["SB"], meta["nchunk"]
    d_out = meta["d_out"]

    ngrp = (ntiles + GRP_TILES - 1) // GRP_TILES
    grp_tiles = [
        list(range(g * GRP_TILES, min((g + 1) * GRP_TILES, ntiles)))
        for g in range(ngrp)
    ]
    max_ch = max(
        int(chunk_base[ts[-1] + 1] - chunk_base[ts[0]]) for ts in grp_tiles
    )

    nc = bacc.Bacc("TRN2", target_bir_lowering=False, num_devices=n_dev,
                  num_swdge_queues=NQ)

    msgs0A_d = nc.dram_tensor("msgs0A", [128, SA], f16,
                              kind="ExternalInput")
    msgs0B_d = nc.dram_tensor("msgs0B", [128, SB], f16,
                              kind="ExternalInput")
    g0ownA_d = nc.dram_tensor("g0ownA", [LO, D_F], f16, kind="ExternalInput")
    g0ownB_d = nc.dram_tensor("g0ownB", [HI, D_F], f16, kind="ExternalInput")
    idxA_d = nc.dram_tensor("idxA", [128, SA // 16], i16, kind="ExternalInput")
    idxB_d = nc.dram_tensor("idxB", [128, SB // 16], i16, kind="ExternalInput")
    dstl_d = nc.dram_tensor("dstl", [128, nchunk], f16, kind="ExternalInput")
    waug_d = [
        nc.dram_tensor(f"waug{l}", [D_F + 1, do], f16, kind="ExternalInput")
        for l, do in enumerate([D_F, D_F, d_out])
    ]
    sigma_d = nc.dram_tensor("sigma", [1, OWN], f16, kind="ExternalInput")
    isd2_d = nc.dram_tensor("isd2", [TILE, ntiles], f32, kind="ExternalInput")
    isdrow_d = nc.dram_tensor("isdrow", [1, OWN], f32, kind="ExternalInput")
    iota_d = nc.dram_tensor("iota", [TILE, TILE], f16, kind="ExternalInput")
    ident_d = nc.dram_tensor("ident", [TILE, TILE], f16, kind="ExternalInput")
    out_d = nc.dram_tensor("out", [1, OWN], f32, kind="ExternalOutput")

    gownA_d = [nc.dram_tensor(f"gownA{l}", [LO, D_F], f16) for l in (1, 2)]
    gownB_d = [nc.dram_tensor(f"gownB{l}", [HI, D_F], f16) for l in (1, 2)]
    gfullA_d = [
        nc.dram_tensor(f"gfullA{l}", [NA, D_F], f16, addr_space="Shared")
        for l in (1, 2)
    ]
    gfullB_d = [
        nc.dram_tensor(f"gfullB{l}", [NB, D_F], f16, addr_space="Shared")
        for l in (1, 2)
    ]

    rg = [list(range(NC_CORES))]

    with TileContext(nc) as tc:
        with (
            tc.tile_pool(name="static", bufs=1) as stp,
            tc.tile_pool(name="msgs", bufs=2 * NQ) as mp,
            tc.tile_pool(name="smat", bufs=3) as sp,
            tc.tile_pool(name="gself", bufs=2) as gp,
            tc.tile_pool(name="paug", bufs=2) as pp,
            tc.tile_pool(name="qrelu", bufs=2) as qp,
            tc.tile_pool(name="gout", bufs=3) as gop,
            tc.tile_pool(name="pps", bufs=2, space="PSUM") as p_ps,
            tc.tile_pool(name="qps", bufs=2, space="PSUM") as q_ps,
            tc.tile_pool(name="tps", bufs=2, space="PSUM") as t_ps,
        ):
            # dma_gather burns one GPSIMD register per distinct num_idxs via
            # to_reg; cache by value so 3 layers x many windows don't exhaust
            # the register file.
            reg_cache = {}
            qn = [0]

            def nreg(v):
                if v not in reg_cache:
                    r = nc.gpsimd.alloc_register(f"nidx{v}")
                    nc.gpsimd.reg_mov(r, v)
                    reg_cache[v] = r
                return reg_cache[v]

            iota_sb = stp.tile([TILE, TILE], f16)
            nc.sync.dma_start(out=iota_sb[:], in_=iota_d[:])
            ident_sb = stp.tile([TILE, TILE], f16)
            nc.sync.dma_start(out=ident_sb[:], in_=ident_d[:])
            ident32_sb = stp.tile([TILE, TILE], f32)
            nc.vector.tensor_copy(ident32_sb[:], ident_sb[:])
            waug_sb = []
            for l, do in enumerate([D_F, D_F, d_out]):
                w = stp.tile([D_F + 1, do], f16, tag=f"waug{l}")
                nc.sync.dma_start(out=w[:], in_=waug_d[l][:])
                waug_sb.append(w)
            isd2_sb = stp.tile([TILE, ntiles], f32)
            nc.sync.dma_start(out=isd2_sb[:], in_=isd2_d[:])
            isdrow_sb = stp.tile([1, OWN], f32)
            nc.sync.dma_start(out=isdrow_sb[:], in_=isdrow_d[:])
            idxA_sb = stp.tile([128, SA // 16], i16)
            nc.sync.dma_start(out=idxA_sb[:], in_=idxA_d[:])
            idxB_sb = stp.tile([128, SB // 16], i16)
            nc.sync.dma_start(out=idxB_sb[:], in_=idxB_d[:])
            dstl_sb = stp.tile([128, nchunk], f16)
            nc.sync.dma_start(out=dstl_sb[:], in_=dstl_d[:])
            out_sb = stp.tile([1, OWN], f32)

            def emit_windows_interleaved(gtabA, gtabB):
                # Both collectives are already dispatched; interleave the two
                # streams' windows so each tile group's A and B chunks land
                # together and the first groups can start as early as possible.
                lstA = _emit_stream("A", idxA_sb, gtabA, NA, collect=False)
                lstB = _emit_stream("B", idxB_sb, gtabB, NB, collect=False)
                outA, outB = [], []
                for i in range(max(len(lstA), len(lstB))):
                    if i < len(lstA):
                        outA.append(lstA[i]())
                    if i < len(lstB):
                        outB.append(lstB[i]())
                return outA, outB

            def _emit_stream(st, idx_sb, gtab, nrows, collect=True):
                thunks = emit_windows(st, idx_sb, gtab, nrows, thunks=True)
                if collect:
                    return [t() for t in thunks]
                return thunks

            def emit_windows(st, idx_sb, gtab, nrows, thunks=False):
                # Each dma_gather covers WIN chunks (the SWDGE ring holds
                # ~1024 descs); windows round-robin the NQ queues so ring
                # drain overlaps desc-gen.  Elements are 256B = 2 rows.
                gslab = gtab[0:nrows, :].rearrange("(a b) f -> a (b f)", b=2)
                nch_st = (SA if st == "A" else SB) // TILE
                lst = []
                for w in range(0, nch_st, WIN):
                    def mk(w=w):
                        kw = min(WIN, nch_st - w)
                        wt = mp.tile([128, WIN * TILE], f16, tag=f"win{st}")
                        nc.gpsimd.dma_gather(
                            wt[:, : kw * TILE].rearrange(
                                "p (c e) -> p c e", e=TILE
                            ),
                            gslab,
                            idx_sb[:, w * 8 : (w + kw) * 8],
                            kw * TILE,
                            nreg(kw * TILE),
                            TILE,
                            queue_num=qn[0],
                        )
                        qn[0] = (qn[0] + 1) % NQ
                        return wt
                    if thunks:
                        lst.append(mk)
                    else:
                        lst.append(mk())
                return lst

            def emit_windows_dram(st, src_d):
                # layer 0: windows are plain HWDGE loads of host-prepacked
                # messages -- zero GPSIMD descriptor generation.
                nch_st = (SA if st == "A" else SB) // TILE
                lst = []
                for w in range(0, nch_st, WIN):
                    kw = min(WIN, nch_st - w)
                    wt = mp.tile([128, WIN * TILE], f16, tag=f"win{st}")
                    nc.sync.dma_start(
                        out=wt[:, : kw * TILE],
                        in_=src_d[:, w * TILE : (w + kw) * TILE],
                    )
                    lst.append(wt)
                return lst

            nlayers = 3 if stage >= 7 else 1  # stage 8: 3 layers, no CC
            if stage < 7:
                nc.vector.memset(out_sb[:], 0.0)
            wins = {
                "A": emit_windows_dram("A", msgs0A_d),
                "B": emit_windows_dram("B", msgs0B_d),
            }
            for layer in range(nlayers):
                gownA_src = [g0ownA_d, gownA_d[0], gownA_d[1]][layer]
                gownB_src = [g0ownB_d, gownB_d[0], gownB_d[1]][layer]
                do = D_F if layer < 2 else d_out

                def msg_lhs(st, chunk, parity):
                    wt = wins[st][chunk // WIN]
                    col = (chunk % WIN) * TILE + parity * D_F
                    return wt[:, col : col + D_F]

                for g, ts in enumerate(grp_tiles):
                    t0, t1 = ts[0], ts[-1] + 1
                    gw = (t1 - t0) * TILE
                    row0 = t0 * TILE
                    rows = min(gw, OWN - row0)
                    c0 = int(chunk_base[t0])
                    nch = int(chunk_base[t1] - c0)

                    # ---- one-hot scatter matrices for every chunk ----------
                    if stage < 2:
                        continue
                    S = sp.tile([128, max_ch * TILE], f16, tag="S")
                    nc.vector.tensor_tensor(
                        S[:, : nch * TILE].rearrange("p (c e) -> p c e", e=TILE),
                        iota_sb[:].unsqueeze(1).broadcast_to([TILE, nch, TILE]),
                        dstl_sb[:, c0 : c0 + nch]
                        .unsqueeze(2)
                        .broadcast_to([TILE, nch, TILE]),
                        mybir.AluOpType.is_equal,
                    )

                    # ---- own-node g rows for the self-loop term ------------
                    if stage < 3:
                        continue
                    gself = gp.tile([TILE, (t1 - t0) * D_F], f16, tag="gself")
                    if rows < gw:
                        nc.vector.memset(gself[:], 0.0)
                    for ti, t in enumerate(ts):
                        r0 = row0 + ti * TILE
                        r = min(TILE, OWN - r0)
                        if r0 < LO:
                            own_src = gownA_src[r0 : r0 + r, :]
                        else:
                            own_src = gownB_src[r0 - LO : r0 - LO + r, :]
                        nc.sync.dma_start(
                            out=gself[0:r, ti * D_F : ti * D_F + D_F],
                            in_=own_src,
                        )

                    # ---- seg-sum into PSUM, one region per dst tile --------
                    ps = p_ps.tile([D_F, gw], f32, space="PSUM", tag="ps")
                    for ti, t in enumerate(ts):
                        sl = slice(ti * TILE, (ti + 1) * TILE)
                        nA0, nA1 = int(C4[t, 0, 0]), int(C4[t, 0, 1])
                        nB0, nB1 = int(C4[t, 1, 0]), int(C4[t, 1, 1])
                        nmm = nA0 + nA1 + nB0 + nB1
                        nc.tensor.matmul(
                            out=ps[:, sl],
                            lhsT=gself[:, ti * D_F : ti * D_F + D_F],
                            rhs=ident_sb[:],
                            start=True,
                            stop=(nmm == 0),
                        )
                        for j in range(nmm):
                            if j < nA0 + nA1:
                                par = int(j >= nA0)
                                lhs = msg_lhs("A", int(a_off[t]) + j, par)
                            else:
                                jb = j - (nA0 + nA1)
                                par = int(jb >= nB0)
                                lhs = msg_lhs("B", int(b_off[t]) + jb, par)
                            scol = (int(chunk_base[t]) - c0 + j) * TILE
                            nc.tensor.matmul(
                                out=ps[:, sl],
                                lhsT=lhs,
                                rhs=S[:, scol : scol + TILE],
                                start=False,
                                stop=(j == nmm - 1),
                            )

                    # ---- augmented dense layer: Q = Waug^T @ [P; sigma] ----
                    if stage < 4:
                        continue
                    paug = pp.tile([D_F + 1, gw], f16, tag="paug")
                    nc.vector.tensor_copy(paug[0:D_F, :gw], ps[:, :gw])
                    nc.sync.dma_start(
                        out=paug[D_F : D_F + 1, 0:rows],
                        in_=sigma_d[:, row0 : row0 + rows],
                    )
                    if rows < gw:
                        nc.vector.memset(paug[D_F : D_F + 1, rows:gw], 0.0)
                    qs = q_ps.tile([D_F, gw], f32, space="PSUM", tag="qs")
                    nc.tensor.matmul(
                        out=qs[0:do, :gw],
                        lhsT=waug_sb[layer][:],
                        rhs=paug[:, :gw],
                        start=True,
                        stop=True,
                    )

                    if stage < 5:
                        continue
                    if layer < 2:
                        # g' = isd^2 * relu(Q), transposed back to node-major
                        qr = qp.tile([D_F, gw], f32, tag="qr")
                        nc.scalar.activation(
                            qr[:, :gw],
                            qs[0:D_F, :gw],
                            mybir.ActivationFunctionType.Relu,
                        )
                        for ti, t in enumerate(ts):
                            qt = t_ps.tile([TILE, D_F], f32, space="PSUM", tag="qt")
                            nc.tensor.transpose(
                                out=qt[:],
                                in_=qr[:, ti * TILE : (ti + 1) * TILE],
                                identity=ident32_sb[0:D_F, 0:D_F],
                            )
                            gsl = gop.tile([TILE, D_F], f16, tag="gsl")
                            nc.vector.tensor_scalar_mul(
                                gsl[:], qt[:], isd2_sb[:, t : t + 1]
                            )
                            r0 = row0 + ti * TILE
                            r = min(TILE, OWN - r0)
                            if r0 < LO:
                                own_dst = gownA_d[layer][r0 : r0 + r, :]
                            else:
                                own_dst = gownB_d[layer][r0 - LO : r0 - LO + r, :]
                            nc.sync.dma_start(out=own_dst, in_=gsl[0:r, :])
                    else:
                        nc.vector.tensor_copy(
                            out_sb[:, row0 : row0 + rows], qs[0:1, 0:rows]
                        )

                if layer < 2 and stage >= 6 and stage != 8:
                    # Dispatch both half-collectives back-to-back (the CC
                    # stream runs them in order), then emit the two gather
                    # streams' windows interleaved so each tile group's A and
                    # B chunks arrive together right after collB lands.
                    nc.gpsimd.collective_compute(
                        "AllGather",
                        mybir.AluOpType.bypass,
                        replica_groups=rg,
                        ins=[gownA_d[layer][:]],
                        outs=[gfullA_d[layer][:]],
                    )
                    nc.gpsimd.collective_compute(
                        "AllGather",
                        mybir.AluOpType.bypass,
                        replica_groups=rg,
                        ins=[gownB_d[layer][:]],
                        outs=[gfullB_d[layer][:]],
                    )
                    winsA, winsB = emit_windows_interleaved(
                        gfullA_d[layer], gfullB_d[layer]
                    )
                    wins = {"A": winsA, "B": winsB}

            # out = isd * Q2  (host reshapes [1, OWN] -> [OWN, 1])
            nc.vector.tensor_tensor(
                out_sb[:], out_sb[:], isdrow_sb[:], mybir.AluOpType.mult
            )
            nc.sync.dma_start(out=out_d[:], in_=out_sb[:])

    nc.compile()
    return nc


# ----------------------------------------------------------------------------
# entry point
# ----------------------------------------------------------------------------


def kernel(x, edge_index, W0, b0, W1, b1, W2, b2):
    from concourse.bass_utils import run_bass_kernel_spmd

    meta, in_maps = _prepare(x, edge_index, W0, b0, W1, b1, W2, b2)
    nc = _build(meta)
    res = run_bass_kernel_spmd(nc, in_maps, list(range(NC_CORES)))
    out = np.concatenate(
        [res.results[c]["out"].reshape(-1, 1) for c in range(NC_CORES)], axis=0
    )
    return out.astype(np.float32)


# revision 12
# speedup vs baseline: 1.3740x; 1.0807x over previous
"""3-layer GCN (GCNConvNet) on 8 Trainium2 NeuronCores.

Math refactor: with isd = 1/sqrt(deg+1) and self-loop edges folded in,
each GCN layer  h' = relu( D^-1/2 (A+I) D^-1/2 (h W^T + 1 b^T) )  becomes

    g      = isd**2 * relu(Q_prev)          (node-major "source features")
    P[n]   = sum_{e: dst(e)=n} g[src(e)]    (+ g[n] self term)
    Q[n]   = Waug^T @ [P[n]; sigma[n]]      (Waug = [W^T; b], sigma = row sums)
    h'     = relu(isd * Q) = isd * relu(Q)  -> g' = isd^2 * relu(Q)

so every per-edge coefficient disappears into per-node scaling and the
scatter matrices are pure one-hot.  The final layer output is isd * Q2.

Sharding: nodes split into 8 contiguous dst ranges (6250 each).  Each core
computes P for its own range over ALL edges.  Edge gathers use
nc.gpsimd.dma_gather (int16 indices) with 256B elements that each cover
TWO unpadded 64-feature fp16 rows of the node-ordered g table; element
index = src//2 < 25000 fits int16 with no table permutation.  Edges are
bucketed by (dst tile, source-row parity) so each 128-edge chunk reads one
64-column half of its gathered window.

Layer 0's message windows are a pure permutation of host-known data
(g0 = isd*x), so they are prepacked on the host and loaded with plain
sync-engine DMAs -- zero GPSIMD descriptor generation for layer 0.  After
layers 0 and 1 a single AllGather (rank order == node order) rebuilds the
full g table.

The per-dst-tile chunk structure is derived from the actual edge data at
kernel() call time and padded to the max over the 8 cores so that all
cores run one shared NEFF (SPMD).
"""

import numpy as np

NC_CORES = 8
TILE = 128
GRP_TILES = 4  # dst tiles fused per PSUM/matmul group (4*128 = 512 <= max N)
D_F = 64  # feature width of hidden layers
WIN = 8  # chunks per gather window (8*128 = 1024 descs = SWDGE ring limit)
NQ = 4  # SWDGE queues


def _wrap16(v):
    """[S] int -> [128, S//16] int16, index i at [i%16, i//16], replicated x8."""
    S = v.shape[0]
    assert S % 16 == 0
    w = v.reshape(S // 16, 16).T.astype(np.int16)
    return np.ascontiguousarray(np.tile(w, (8, 1)))


def _prepare(x, edge_index, W0, b0, W1, b1, W2, b2):
    x = np.asarray(x, dtype=np.float32)
    ei = np.asarray(edge_index)
    W0 = np.asarray(W0, np.float32)
    b0 = np.asarray(b0, np.float32)
    W1 = np.asarray(W1, np.float32)
    b1 = np.asarray(b1, np.float32)
    W2 = np.asarray(W2, np.float32)
    b2 = np.asarray(b2, np.float32)

    N = x.shape[0]
    assert N % NC_CORES == 0
    OWN = N // NC_CORES
    assert OWN % 2 == 0
    ntiles = (OWN + TILE - 1) // TILE
    assert N // 2 <= 32768, "int16 element indices"
    src = ei[0].astype(np.int64)
    dst = ei[1].astype(np.int64)

    deg = np.bincount(dst, minlength=N).astype(np.float32) + 1.0
    isd = (1.0 / np.sqrt(deg)).astype(np.float32)
    sigma = (
        np.bincount(dst, weights=isd[src].astype(np.float64), minlength=N).astype(
            np.float32
        )
        + isd
    )

    g0 = (isd[:, None] * x).astype(np.float16)  # [N, 64] unpadded, node order

    # ---- edge bucketing: (core, tile, parity) -------------------------------
    par = src % 2
    elem = src // 2
    core = dst // OWN
    tl = (dst % OWN) // TILE
    key = ((core * ntiles + tl) * 2) + par
    order = np.argsort(key, kind="stable")
    s_elem = elem[order]
    s_dstl = (dst % OWN) % TILE
    s_dstl = s_dstl[order]
    counts = np.bincount(key, minlength=NC_CORES * ntiles * 2).reshape(
        NC_CORES, ntiles, 2
    )
    starts = np.zeros(NC_CORES * ntiles * 2 + 1, np.int64)
    np.cumsum(counts.reshape(-1), out=starts[1:])

    # chunks per (tile, parity), shared across cores
    C2 = np.maximum(1, -(-counts.max(axis=0) // TILE)).astype(np.int64)
    n_t = C2[:, 0] + C2[:, 1]

    a_off = np.zeros(ntiles + 1, np.int64)  # chunk offsets into the stream
    np.cumsum(n_t, out=a_off[1:])
    SA = int(a_off[-1]) * TILE
    nchunk = int(a_off[-1])

    per_core = []
    for c in range(NC_CORES):
        # pad slots must gather *something*; spread them over distinct
        # elements so they don't hammer one HBM line (S-col is -1 so the
        # gathered values never contribute).
        sA = np.arange(SA, dtype=np.int64) % (N // 2)
        dstl_flat = np.full(nchunk * TILE, -1.0, np.float32)
        for t in range(ntiles):
            slot = a_off[t] * TILE
            for p in range(2):
                k = (c * ntiles + t) * 2 + p
                lo, hi = starts[k], starts[k + 1]
                n = hi - lo
                sA[slot : slot + n] = s_elem[lo:hi]
                dstl_flat[slot : slot + n] = s_dstl[lo:hi]
                slot += C2[t, p] * TILE
        # layer-0 message windows: prepacked host-side (pure permutation)
        m0 = g0.reshape(N // 2, 2 * D_F)[sA]
        m0 = np.ascontiguousarray(
            m0.reshape(SA // TILE, TILE, 2 * D_F).transpose(1, 0, 2).reshape(
                TILE, SA
            )
        )
        own = isd[c * OWN : (c + 1) * OWN] ** 2
        tmp = np.zeros(ntiles * TILE, np.float32)
        tmp[:OWN] = own
        isd2 = np.ascontiguousarray(tmp.reshape(ntiles, TILE).T)
        per_core.append(
            dict(
                idxA=_wrap16(sA),
                msgs0=m0,
                dstl=np.ascontiguousarray(
                    dstl_flat.reshape(nchunk, TILE).T.astype(np.float16)
                ),
                sigma=sigma[c * OWN : (c + 1) * OWN]
                .astype(np.float16)
                .reshape(1, OWN),
                isd2=isd2,
                isdrow=isd[c * OWN : (c + 1) * OWN]
                .astype(np.float32)
                .reshape(1, OWN),
                g0own=np.ascontiguousarray(g0[c * OWN : (c + 1) * OWN]),
            )
        )

    waug = []
    for W, b in ((W0, b0), (W1, b1), (W2, b2)):
        wa = np.zeros((D_F + 1, W.shape[0]), np.float16)
        wa[:D_F, :] = W.T.astype(np.float16)
        wa[D_F, :] = b.astype(np.float16)
        waug.append(wa)

    iota = np.tile(np.arange(TILE, dtype=np.float16), (TILE, 1))
    ident = np.eye(TILE, dtype=np.float16)

    meta = dict(
        N=N,
        OWN=OWN,
        ntiles=ntiles,
        C2=C2,
        a_off=a_off,
        SA=SA,
        nchunk=nchunk,
        d_out=W2.shape[0],
    )

    in_maps = []
    for c in range(NC_CORES):
        m = dict(per_core[c])
        m["waug0"] = waug[0]
        m["waug1"] = waug[1]
        m["waug2"] = waug[2]
        m["iota"] = iota
        m["ident"] = ident
        in_maps.append(m)
    return meta, in_maps


def _build(meta, stage=99, n_dev=NC_CORES):
    import concourse.bacc as bacc
    import concourse.mybir as mybir
    from concourse.tile import TileContext

    f16 = mybir.dt.float16
    f32 = mybir.dt.float32
    i16 = mybir.dt.int16

    N = meta["N"]
    OWN = meta["OWN"]
    ntiles = meta["ntiles"]
    C2 = meta["C2"]
    a_off = meta["a_off"]
    SA, nchunk = meta["SA"], meta["nchunk"]
    d_out = meta["d_out"]

    ngrp = (ntiles + GRP_TILES - 1) // GRP_TILES
    grp_tiles = [
        list(range(g * GRP_TILES, min((g + 1) * GRP_TILES, ntiles)))
        for g in range(ngrp)
    ]
    max_ch = max(int(a_off[ts[-1] + 1] - a_off[ts[0]]) for ts in grp_tiles)

    nc = bacc.Bacc("TRN2", target_bir_lowering=False, num_devices=n_dev,
                  num_swdge_queues=NQ)

    msgs0_d = nc.dram_tensor("msgs0", [128, SA], f16, kind="ExternalInput")
    g0own_d = nc.dram_tensor("g0own", [OWN, D_F], f16, kind="ExternalInput")
    idxA_d = nc.dram_tensor("idxA", [128, SA // 16], i16, kind="ExternalInput")
    dstl_d = nc.dram_tensor("dstl", [128, nchunk], f16, kind="ExternalInput")
    waug_d = [
        nc.dram_tensor(f"waug{l}", [D_F + 1, do], f16, kind="ExternalInput")
        for l, do in enumerate([D_F, D_F, d_out])
    ]
    sigma_d = nc.dram_tensor("sigma", [1, OWN], f16, kind="ExternalInput")
    isd2_d = nc.dram_tensor("isd2", [TILE, ntiles], f32, kind="ExternalInput")
    isdrow_d = nc.dram_tensor("isdrow", [1, OWN], f32, kind="ExternalInput")
    iota_d = nc.dram_tensor("iota", [TILE, TILE], f16, kind="ExternalInput")
    ident_d = nc.dram_tensor("ident", [TILE, TILE], f16, kind="ExternalInput")
    out_d = nc.dram_tensor("out", [1, OWN], f32, kind="ExternalOutput")

    gown_d = [nc.dram_tensor(f"gown{l}", [OWN, D_F], f16) for l in (1, 2)]
    gfull_d = [
        nc.dram_tensor(f"gfull{l}", [N, D_F], f16, addr_space="Shared")
        for l in (1, 2)
    ]

    rg = [list(range(NC_CORES))]

    with TileContext(nc) as tc:
        with (
            tc.tile_pool(name="static", bufs=1) as stp,
            tc.tile_pool(name="msgs", bufs=2 * NQ) as mp,
            tc.tile_pool(name="smat", bufs=3) as sp,
            tc.tile_pool(name="gself", bufs=2) as gp,
            tc.tile_pool(name="paug", bufs=2) as pp,
            tc.tile_pool(name="qrelu", bufs=2) as qp,
            tc.tile_pool(name="gout", bufs=3) as gop,
            tc.tile_pool(name="pps", bufs=2, space="PSUM") as p_ps,
            tc.tile_pool(name="qps", bufs=2, space="PSUM") as q_ps,
            tc.tile_pool(name="tps", bufs=2, space="PSUM") as t_ps,
        ):
            reg_cache = {}
            qn = [0]

            def nreg(v):
                if v not in reg_cache:
                    r = nc.gpsimd.alloc_register(f"nidx{v}")
                    nc.gpsimd.reg_mov(r, v)
                    reg_cache[v] = r
                return reg_cache[v]

            iota_sb = stp.tile([TILE, TILE], f16)
            nc.sync.dma_start(out=iota_sb[:], in_=iota_d[:])
            ident_sb = stp.tile([TILE, TILE], f16)
            nc.sync.dma_start(out=ident_sb[:], in_=ident_d[:])
            ident32_sb = stp.tile([TILE, TILE], f32)
            nc.vector.tensor_copy(ident32_sb[:], ident_sb[:])
            waug_sb = []
            for l, do in enumerate([D_F, D_F, d_out]):
                w = stp.tile([D_F + 1, do], f16, tag=f"waug{l}")
                nc.sync.dma_start(out=w[:], in_=waug_d[l][:])
                waug_sb.append(w)
            isd2_sb = stp.tile([TILE, ntiles], f32)
            nc.sync.dma_start(out=isd2_sb[:], in_=isd2_d[:])
            isdrow_sb = stp.tile([1, OWN], f32)
            nc.sync.dma_start(out=isdrow_sb[:], in_=isdrow_d[:])
            idxA_sb = stp.tile([128, SA // 16], i16)
            nc.sync.dma_start(out=idxA_sb[:], in_=idxA_d[:])
            dstl_sb = stp.tile([128, nchunk], f16)
            nc.sync.dma_start(out=dstl_sb[:], in_=dstl_d[:])
            out_sb = stp.tile([1, OWN], f32)

            nch_all = SA // TILE

            def emit_windows_gather(gtab):
                gslab = gtab[0:N, :].rearrange("(a b) f -> a (b f)", b=2)
                lst = []
                for w in range(0, nch_all, WIN):
                    kw = min(WIN, nch_all - w)
                    wt = mp.tile([128, WIN * TILE], f16, tag="win")
                    nc.gpsimd.dma_gather(
                        wt[:, : kw * TILE].rearrange("p (c e) -> p c e", e=TILE),
                        gslab,
                        idxA_sb[:, w * 8 : (w + kw) * 8],
                        kw * TILE,
                        nreg(kw * TILE),
                        TILE,
                        queue_num=qn[0],
                    )
                    qn[0] = (qn[0] + 1) % NQ
                    lst.append(wt)
                return lst

            def emit_windows_dram():
                # layer 0: plain HWDGE loads of host-prepacked messages
                lst = []
                for w in range(0, nch_all, WIN):
                    kw = min(WIN, nch_all - w)
                    wt = mp.tile([128, WIN * TILE], f16, tag="win")
                    nc.sync.dma_start(
                        out=wt[:, : kw * TILE],
                        in_=msgs0_d[:, w * TILE : (w + kw) * TILE],
                    )
                    lst.append(wt)
                return lst

            nlayers = 3 if stage >= 7 else 1
            if stage < 7:
                nc.vector.memset(out_sb[:], 0.0)
            wins = emit_windows_dram()
            for layer in range(nlayers):
                gown_src = [g0own_d, gown_d[0], gown_d[1]][layer]
                do = D_F if layer < 2 else d_out

                def msg_lhs(chunk, parity):
                    wt = wins[chunk // WIN]
                    col = (chunk % WIN) * TILE + parity * D_F
                    return wt[:, col : col + D_F]

                for g, ts in enumerate(grp_tiles):
                    t0, t1 = ts[0], ts[-1] + 1
                    gw = (t1 - t0) * TILE
                    row0 = t0 * TILE
                    rows = min(gw, OWN - row0)
                    c0 = int(a_off[t0])
                    nch = int(a_off[t1] - c0)

                    if stage < 2:
                        continue
                    S = sp.tile([128, max_ch * TILE], f16, tag="S")
                    nc.vector.tensor_tensor(
                        S[:, : nch * TILE].rearrange("p (c e) -> p c e", e=TILE),
                        iota_sb[:].unsqueeze(1).broadcast_to([TILE, nch, TILE]),
                        dstl_sb[:, c0 : c0 + nch]
                        .unsqueeze(2)
                        .broadcast_to([TILE, nch, TILE]),
                        mybir.AluOpType.is_equal,
                    )

                    if stage < 3:
                        continue
                    gself = gp.tile([TILE, (t1 - t0) * D_F], f16, tag="gself")
                    if rows < gw:
                        nc.vector.memset(gself[:], 0.0)
                    for ti, t in enumerate(ts):
                        r0 = row0 + ti * TILE
                        r = min(TILE, OWN - r0)
                        nc.sync.dma_start(
                            out=gself[0:r, ti * D_F : ti * D_F + D_F],
                            in_=gown_src[r0 : r0 + r, :],
                        )

                    ps = p_ps.tile([D_F, gw], f32, space="PSUM", tag="ps")
                    for ti, t in enumerate(ts):
                        sl = slice(ti * TILE, (ti + 1) * TILE)
                        n0, n1 = int(C2[t, 0]), int(C2[t, 1])
                        nmm = n0 + n1
                        nc.tensor.matmul(
                            out=ps[:, sl],
                            lhsT=gself[:, ti * D_F : ti * D_F + D_F],
                            rhs=ident_sb[:],
                            start=True,
                            stop=(nmm == 0),
                        )
                        for j in range(nmm):
                            par = int(j >= n0)
                            lhs = msg_lhs(int(a_off[t]) + j, par)
                            scol = (int(a_off[t]) - c0 + j) * TILE
                            nc.tensor.matmul(
                                out=ps[:, sl],
                                lhsT=lhs,
                                rhs=S[:, scol : scol + TILE],
                                start=False,
                                stop=(j == nmm - 1),
                            )

                    if stage < 4:
                        continue
                    paug = pp.tile([D_F + 1, gw], f16, tag="paug")
                    nc.vector.tensor_copy(paug[0:D_F, :gw], ps[:, :gw])
                    nc.sync.dma_start(
                        out=paug[D_F : D_F + 1, 0:rows],
                        in_=sigma_d[:, row0 : row0 + rows],
                    )
                    if rows < gw:
                        nc.vector.memset(paug[D_F : D_F + 1, rows:gw], 0.0)
                    qs = q_ps.tile([D_F, gw], f32, space="PSUM", tag="qs")
                    nc.tensor.matmul(
                        out=qs[0:do, :gw],
                        lhsT=waug_sb[layer][:],
                        rhs=paug[:, :gw],
                        start=True,
                        stop=True,
                    )

                    if stage < 5:
                        continue
                    if layer < 2:
                        qr = qp.tile([D_F, gw], f32, tag="qr")
                        nc.scalar.activation(
                            qr[:, :gw],
                            qs[0:D_F, :gw],
                            mybir.ActivationFunctionType.Relu,
                        )
                        for ti, t in enumerate(ts):
                            qt = t_ps.tile([TILE, D_F], f32, space="PSUM", tag="qt")
                            nc.tensor.transpose(
                                out=qt[:],
                                in_=qr[:, ti * TILE : (ti + 1) * TILE],
                                identity=ident32_sb[0:D_F, 0:D_F],
                            )
                            gsl = gop.tile([TILE, D_F], f16, tag="gsl")
                            nc.vector.tensor_scalar_mul(
                                gsl[:], qt[:], isd2_sb[:, t : t + 1]
                            )
                            r0 = row0 + ti * TILE
                            r = min(TILE, OWN - r0)
                            nc.sync.dma_start(
                                out=gown_d[layer][r0 : r0 + r, :],
                                in_=gsl[0:r, :],
                            )
                    else:
                        nc.vector.tensor_copy(
                            out_sb[:, row0 : row0 + rows], qs[0:1, 0:rows]
                        )

                if layer < 2 and stage >= 6 and stage != 8:
                    nc.gpsimd.collective_compute(
                        "AllGather",
                        mybir.AluOpType.bypass,
                        replica_groups=rg,
                        ins=[gown_d[layer][:]],
                        outs=[gfull_d[layer][:]],
                    )
                    wins = emit_windows_gather(gfull_d[layer])

            nc.vector.tensor_tensor(
                out_sb[:], out_sb[:], isdrow_sb[:], mybir.AluOpType.mult
            )
            nc.sync.dma_start(out=out_d[:], in_=out_sb[:])

    nc.compile()
    return nc


def kernel(x, edge_index, W0, b0, W1, b1, W2, b2):
    from concourse.bass_utils import run_bass_kernel_spmd

    meta, in_maps = _prepare(x, edge_index, W0, b0, W1, b1, W2, b2)
    nc = _build(meta)
    res = run_bass_kernel_spmd(nc, in_maps, list(range(NC_CORES)))
    out = np.concatenate(
        [res.results[c]["out"].reshape(-1, 1) for c in range(NC_CORES)], axis=0
    )
    return out.astype(np.float32)


# revision 13
# speedup vs baseline: 1.3962x; 1.0162x over previous
"""3-layer GCN (GCNConvNet) on 8 Trainium2 NeuronCores.

Math refactor: with isd = 1/sqrt(deg+1) and self-loop edges folded in,
each GCN layer  h' = relu( D^-1/2 (A+I) D^-1/2 (h W^T + 1 b^T) )  becomes

    g      = isd**2 * relu(Q_prev)          (node-major "source features")
    P[n]   = sum_{e: dst(e)=n} g[src(e)]    (+ g[n] self term)
    Q[n]   = Waug^T @ [P[n]; sigma[n]]      (Waug = [W^T; b], sigma = row sums)
    h'     = relu(isd * Q) = isd * relu(Q)  -> g' = isd^2 * relu(Q)

so every per-edge coefficient disappears into per-node scaling and the
scatter matrices are pure one-hot.  The final layer output is isd * Q2.

Sharding: nodes split into 8 contiguous dst ranges (6250 each).  Each core
computes P for its own range over ALL edges.  Edge gathers use
nc.gpsimd.dma_gather (int16 indices) with 256B elements that each cover
TWO unpadded 64-feature fp16 rows of the node-ordered g table; element
index = src//2 < 25000 fits int16 with no table permutation.  Edges are
bucketed by (dst tile, source-row parity) so each 128-edge chunk reads one
64-column half of its gathered window.

Layer 0's message windows are a pure permutation of host-known data
(g0 = isd*x), so they are prepacked on the host and loaded with plain
sync-engine DMAs -- zero GPSIMD descriptor generation for layer 0.  After
layers 0 and 1 a single AllGather (rank order == node order) rebuilds the
full g table.

The per-dst-tile chunk structure is derived from the actual edge data at
kernel() call time and padded to the max over the 8 cores so that all
cores run one shared NEFF (SPMD).
"""

import numpy as np

NC_CORES = 8
TILE = 128
GRP_TILES = 4  # dst tiles fused per PSUM/matmul group (4*128 = 512 <= max N)
D_F = 64  # feature width of hidden layers
WIN = 8  # chunks per gather window (8*128 = 1024 descs = SWDGE ring limit)
NQ = 4  # SWDGE queues


def _wrap16(v):
    """[S] int -> [128, S//16] int16, index i at [i%16, i//16], replicated x8."""
    S = v.shape[0]
    assert S % 16 == 0
    w = v.reshape(S // 16, 16).T.astype(np.int16)
    return np.ascontiguousarray(np.tile(w, (8, 1)))


def _prepare(x, edge_index, W0, b0, W1, b1, W2, b2):
    x = np.asarray(x, dtype=np.float32)
    ei = np.asarray(edge_index)
    W0 = np.asarray(W0, np.float32)
    b0 = np.asarray(b0, np.float32)
    W1 = np.asarray(W1, np.float32)
    b1 = np.asarray(b1, np.float32)
    W2 = np.asarray(W2, np.float32)
    b2 = np.asarray(b2, np.float32)

    N = x.shape[0]
    assert N % NC_CORES == 0
    OWN = N // NC_CORES
    assert OWN % 2 == 0
    ntiles = (OWN + TILE - 1) // TILE
    assert N // 2 <= 32768, "int16 element indices"
    src = ei[0].astype(np.int64)
    dst = ei[1].astype(np.int64)

    deg = np.bincount(dst, minlength=N).astype(np.float32) + 1.0
    isd = (1.0 / np.sqrt(deg)).astype(np.float32)
    sigma = (
        np.bincount(dst, weights=isd[src].astype(np.float64), minlength=N).astype(
            np.float32
        )
        + isd
    )

    g0 = (isd[:, None] * x).astype(np.float16)  # [N, 64] unpadded, node order

    # ---- edge bucketing: (core, tile, parity) -------------------------------
    par = src % 2
    elem = src // 2
    core = dst // OWN
    tl = (dst % OWN) // TILE
    key = ((core * ntiles + tl) * 2) + par
    order = np.argsort(key, kind="stable")
    s_elem = elem[order]
    s_dstl = (dst % OWN) % TILE
    s_dstl = s_dstl[order]
    counts = np.bincount(key, minlength=NC_CORES * ntiles * 2).reshape(
        NC_CORES, ntiles, 2
    )
    starts = np.zeros(NC_CORES * ntiles * 2 + 1, np.int64)
    np.cumsum(counts.reshape(-1), out=starts[1:])

    # chunks per (tile, parity), shared across cores
    C2 = np.maximum(1, -(-counts.max(axis=0) // TILE)).astype(np.int64)
    n_t = C2[:, 0] + C2[:, 1]

    a_off = np.zeros(ntiles + 1, np.int64)  # chunk offsets into the stream
    np.cumsum(n_t, out=a_off[1:])
    SA = int(a_off[-1]) * TILE
    nchunk = int(a_off[-1])

    per_core = []
    for c in range(NC_CORES):
        # pad slots must gather *something*; spread them over distinct
        # elements so they don't hammer one HBM line (S-col is -1 so the
        # gathered values never contribute).
        sA = np.arange(SA, dtype=np.int64) % (N // 2)
        dstl_flat = np.full(nchunk * TILE, -1.0, np.float32)
        for t in range(ntiles):
            slot = a_off[t] * TILE
            for p in range(2):
                k = (c * ntiles + t) * 2 + p
                lo, hi = starts[k], starts[k + 1]
                n = hi - lo
                sA[slot : slot + n] = s_elem[lo:hi]
                dstl_flat[slot : slot + n] = s_dstl[lo:hi]
                slot += C2[t, p] * TILE
        # layer-0 message windows: prepacked host-side (pure permutation)
        m0 = g0.reshape(N // 2, 2 * D_F)[sA]
        m0 = np.ascontiguousarray(
            m0.reshape(SA // TILE, TILE, 2 * D_F).transpose(1, 0, 2).reshape(
                TILE, SA
            )
        )
        own = isd[c * OWN : (c + 1) * OWN] ** 2
        tmp = np.zeros(ntiles * TILE, np.float32)
        tmp[:OWN] = own
        isd2 = np.ascontiguousarray(tmp.reshape(ntiles, TILE).T)
        per_core.append(
            dict(
                idxA=_wrap16(sA),
                msgs0=m0,
                dstl=np.ascontiguousarray(
                    dstl_flat.reshape(nchunk, TILE).T.astype(np.float16)
                ),
                sigma=sigma[c * OWN : (c + 1) * OWN]
                .astype(np.float16)
                .reshape(1, OWN),
                isd2=isd2,
                isdrow=isd[c * OWN : (c + 1) * OWN]
                .astype(np.float32)
                .reshape(1, OWN),
                g0own=np.ascontiguousarray(g0[c * OWN : (c + 1) * OWN]),
            )
        )

    waug = []
    for W, b in ((W0, b0), (W1, b1), (W2, b2)):
        wa = np.zeros((D_F + 1, W.shape[0]), np.float16)
        wa[:D_F, :] = W.T.astype(np.float16)
        wa[D_F, :] = b.astype(np.float16)
        waug.append(wa)

    iota = np.tile(np.arange(TILE, dtype=np.float16), (TILE, 1))
    ident = np.eye(TILE, dtype=np.float16)

    meta = dict(
        N=N,
        OWN=OWN,
        ntiles=ntiles,
        C2=C2,
        a_off=a_off,
        SA=SA,
        nchunk=nchunk,
        d_out=W2.shape[0],
    )

    in_maps = []
    for c in range(NC_CORES):
        m = dict(per_core[c])
        m["waug0"] = waug[0]
        m["waug1"] = waug[1]
        m["waug2"] = waug[2]
        m["iota"] = iota
        m["ident"] = ident
        in_maps.append(m)
    return meta, in_maps


def _build(meta, stage=99, n_dev=NC_CORES):
    import concourse.bacc as bacc
    import concourse.mybir as mybir
    from concourse.tile import TileContext

    f16 = mybir.dt.float16
    f32 = mybir.dt.float32
    i16 = mybir.dt.int16

    N = meta["N"]
    OWN = meta["OWN"]
    ntiles = meta["ntiles"]
    C2 = meta["C2"]
    a_off = meta["a_off"]
    SA, nchunk = meta["SA"], meta["nchunk"]
    d_out = meta["d_out"]

    ngrp = (ntiles + GRP_TILES - 1) // GRP_TILES
    grp_tiles = [
        list(range(g * GRP_TILES, min((g + 1) * GRP_TILES, ntiles)))
        for g in range(ngrp)
    ]
    max_ch = max(int(a_off[ts[-1] + 1] - a_off[ts[0]]) for ts in grp_tiles)

    nc = bacc.Bacc("TRN2", target_bir_lowering=False, num_devices=n_dev,
                  num_swdge_queues=NQ)

    msgs0_d = nc.dram_tensor("msgs0", [128, SA], f16, kind="ExternalInput")
    g0own_d = nc.dram_tensor("g0own", [OWN, D_F], f16, kind="ExternalInput")
    idxA_d = nc.dram_tensor("idxA", [128, SA // 16], i16, kind="ExternalInput")
    dstl_d = nc.dram_tensor("dstl", [128, nchunk], f16, kind="ExternalInput")
    waug_d = [
        nc.dram_tensor(f"waug{l}", [D_F + 1, do], f16, kind="ExternalInput")
        for l, do in enumerate([D_F, D_F, d_out])
    ]
    sigma_d = nc.dram_tensor("sigma", [1, OWN], f16, kind="ExternalInput")
    isd2_d = nc.dram_tensor("isd2", [TILE, ntiles], f32, kind="ExternalInput")
    isdrow_d = nc.dram_tensor("isdrow", [1, OWN], f32, kind="ExternalInput")
    iota_d = nc.dram_tensor("iota", [TILE, TILE], f16, kind="ExternalInput")
    ident_d = nc.dram_tensor("ident", [TILE, TILE], f16, kind="ExternalInput")
    out_d = nc.dram_tensor("out", [1, OWN], f32, kind="ExternalOutput")

    gown_d = [nc.dram_tensor(f"gown{l}", [OWN, D_F], f16) for l in (1, 2)]
    gfull_d = [
        nc.dram_tensor(f"gfull{l}", [N, D_F], f16, addr_space="Shared")
        for l in (1, 2)
    ]

    rg = [list(range(NC_CORES))]

    with TileContext(nc) as tc:
        with (
            tc.tile_pool(name="static", bufs=1) as stp,
            tc.tile_pool(name="msgs", bufs=10) as mp,
            tc.tile_pool(name="smat", bufs=4) as sp,
            tc.tile_pool(name="gself", bufs=3) as gp,
            tc.tile_pool(name="paug", bufs=2) as pp,
            tc.tile_pool(name="qrelu", bufs=2) as qp,
            tc.tile_pool(name="gout", bufs=3) as gop,
            tc.tile_pool(name="pps", bufs=3, space="PSUM") as p_ps,
            tc.tile_pool(name="qps", bufs=2, space="PSUM") as q_ps,
            tc.tile_pool(name="tps", bufs=2, space="PSUM") as t_ps,
        ):
            reg_cache = {}
            qn = [0]

            def nreg(v):
                if v not in reg_cache:
                    r = nc.gpsimd.alloc_register(f"nidx{v}")
                    nc.gpsimd.reg_mov(r, v)
                    reg_cache[v] = r
                return reg_cache[v]

            iota_sb = stp.tile([TILE, TILE], f16)
            nc.sync.dma_start(out=iota_sb[:], in_=iota_d[:])
            ident_sb = stp.tile([TILE, TILE], f16)
            nc.sync.dma_start(out=ident_sb[:], in_=ident_d[:])
            ident32_sb = stp.tile([TILE, TILE], f32)
            nc.vector.tensor_copy(ident32_sb[:], ident_sb[:])
            waug_sb = []
            for l, do in enumerate([D_F, D_F, d_out]):
                w = stp.tile([D_F + 1, do], f16, tag=f"waug{l}")
                nc.sync.dma_start(out=w[:], in_=waug_d[l][:])
                waug_sb.append(w)
            isd2_sb = stp.tile([TILE, ntiles], f32)
            nc.sync.dma_start(out=isd2_sb[:], in_=isd2_d[:])
            isdrow_sb = stp.tile([1, OWN], f32)
            nc.sync.dma_start(out=isdrow_sb[:], in_=isdrow_d[:])
            idxA_sb = stp.tile([128, SA // 16], i16)
            nc.sync.dma_start(out=idxA_sb[:], in_=idxA_d[:])
            dstl_sb = stp.tile([128, nchunk], f16)
            nc.sync.dma_start(out=dstl_sb[:], in_=dstl_d[:])
            out_sb = stp.tile([1, OWN], f32)

            nch_all = SA // TILE

            def emit_windows_gather(gtab):
                gslab = gtab[0:N, :].rearrange("(a b) f -> a (b f)", b=2)
                lst = []
                for w in range(0, nch_all, WIN):
                    kw = min(WIN, nch_all - w)
                    wt = mp.tile([128, WIN * TILE], f16, tag="win")
                    nc.gpsimd.dma_gather(
                        wt[:, : kw * TILE].rearrange("p (c e) -> p c e", e=TILE),
                        gslab,
                        idxA_sb[:, w * 8 : (w + kw) * 8],
                        kw * TILE,
                        nreg(kw * TILE),
                        TILE,
                        queue_num=qn[0],
                    )
                    qn[0] = (qn[0] + 1) % NQ
                    lst.append(wt)
                return lst

            def emit_windows_dram():
                # layer 0: plain HWDGE loads of host-prepacked messages
                lst = []
                for w in range(0, nch_all, WIN):
                    kw = min(WIN, nch_all - w)
                    wt = mp.tile([128, WIN * TILE], f16, tag="win")
                    nc.sync.dma_start(
                        out=wt[:, : kw * TILE],
                        in_=msgs0_d[:, w * TILE : (w + kw) * TILE],
                    )
                    lst.append(wt)
                return lst

            nlayers = 3 if stage >= 7 else 1
            if stage < 7:
                nc.vector.memset(out_sb[:], 0.0)
            wins = emit_windows_dram()
            for layer in range(nlayers):
                gown_src = [g0own_d, gown_d[0], gown_d[1]][layer]
                do = D_F if layer < 2 else d_out

                def msg_lhs(chunk, parity):
                    wt = wins[chunk // WIN]
                    col = (chunk % WIN) * TILE + parity * D_F
                    return wt[:, col : col + D_F]

                for g, ts in enumerate(grp_tiles):
                    t0, t1 = ts[0], ts[-1] + 1
                    gw = (t1 - t0) * TILE
                    row0 = t0 * TILE
                    rows = min(gw, OWN - row0)
                    c0 = int(a_off[t0])
                    nch = int(a_off[t1] - c0)

                    if stage < 2:
                        continue
                    S = sp.tile([128, max_ch * TILE], f16, tag="S")
                    nc.vector.tensor_tensor(
                        S[:, : nch * TILE].rearrange("p (c e) -> p c e", e=TILE),
                        iota_sb[:].unsqueeze(1).broadcast_to([TILE, nch, TILE]),
                        dstl_sb[:, c0 : c0 + nch]
                        .unsqueeze(2)
                        .broadcast_to([TILE, nch, TILE]),
                        mybir.AluOpType.is_equal,
                    )

                    if stage < 3:
                        continue
                    gself = gp.tile([TILE, (t1 - t0) * D_F], f16, tag="gself")
                    if rows < gw:
                        nc.vector.memset(gself[:], 0.0)
                    for ti, t in enumerate(ts):
                        r0 = row0 + ti * TILE
                        r = min(TILE, OWN - r0)
                        nc.sync.dma_start(
                            out=gself[0:r, ti * D_F : ti * D_F + D_F],
                            in_=gown_src[r0 : r0 + r, :],
                        )

                    ps = p_ps.tile([D_F, gw], f32, space="PSUM", tag="ps")
                    for ti, t in enumerate(ts):
                        sl = slice(ti * TILE, (ti + 1) * TILE)
                        n0, n1 = int(C2[t, 0]), int(C2[t, 1])
                        nmm = n0 + n1
                        nc.tensor.matmul(
                            out=ps[:, sl],
                            lhsT=gself[:, ti * D_F : ti * D_F + D_F],
                            rhs=ident_sb[:],
                            start=True,
                            stop=(nmm == 0),
                        )
                        for j in range(nmm):
                            par = int(j >= n0)
                            lhs = msg_lhs(int(a_off[t]) + j, par)
                            scol = (int(a_off[t]) - c0 + j) * TILE
                            nc.tensor.matmul(
                                out=ps[:, sl],
                                lhsT=lhs,
                                rhs=S[:, scol : scol + TILE],
                                start=False,
                                stop=(j == nmm - 1),
                            )

                    if stage < 4:
                        continue
                    paug = pp.tile([D_F + 1, gw], f16, tag="paug")
                    nc.vector.tensor_copy(paug[0:D_F, :gw], ps[:, :gw])
                    nc.sync.dma_start(
                        out=paug[D_F : D_F + 1, 0:rows],
                        in_=sigma_d[:, row0 : row0 + rows],
                    )
                    if rows < gw:
                        nc.vector.memset(paug[D_F : D_F + 1, rows:gw], 0.0)
                    qs = q_ps.tile([D_F, gw], f32, space="PSUM", tag="qs")
                    nc.tensor.matmul(
                        out=qs[0:do, :gw],
                        lhsT=waug_sb[layer][:],
                        rhs=paug[:, :gw],
                        start=True,
                        stop=True,
                    )

                    if stage < 5:
                        continue
                    if layer < 2:
                        qr = qp.tile([D_F, gw], f32, tag="qr")
                        nc.scalar.activation(
                            qr[:, :gw],
                            qs[0:D_F, :gw],
                            mybir.ActivationFunctionType.Relu,
                        )
                        for ti, t in enumerate(ts):
                            qt = t_ps.tile([TILE, D_F], f32, space="PSUM", tag="qt")
                            nc.tensor.transpose(
                                out=qt[:],
                                in_=qr[:, ti * TILE : (ti + 1) * TILE],
                                identity=ident32_sb[0:D_F, 0:D_F],
                            )
                            gsl = gop.tile([TILE, D_F], f16, tag="gsl")
                            nc.vector.tensor_scalar_mul(
                                gsl[:], qt[:], isd2_sb[:, t : t + 1]
                            )
                            r0 = row0 + ti * TILE
                            r = min(TILE, OWN - r0)
                            nc.sync.dma_start(
                                out=gown_d[layer][r0 : r0 + r, :],
                                in_=gsl[0:r, :],
                            )
                    else:
                        nc.vector.tensor_copy(
                            out_sb[:, row0 : row0 + rows], qs[0:1, 0:rows]
                        )

                if layer < 2 and stage >= 6 and stage != 8:
                    nc.gpsimd.collective_compute(
                        "AllGather",
                        mybir.AluOpType.bypass,
                        replica_groups=rg,
                        ins=[gown_d[layer][:]],
                        outs=[gfull_d[layer][:]],
                    )
                    wins = emit_windows_gather(gfull_d[layer])

            nc.vector.tensor_tensor(
                out_sb[:], out_sb[:], isdrow_sb[:], mybir.AluOpType.mult
            )
            nc.sync.dma_start(out=out_d[:], in_=out_sb[:])

    nc.compile()
    return nc


def kernel(x, edge_index, W0, b0, W1, b1, W2, b2):
    from concourse.bass_utils import run_bass_kernel_spmd

    meta, in_maps = _prepare(x, edge_index, W0, b0, W1, b1, W2, b2)
    nc = _build(meta)
    res = run_bass_kernel_spmd(nc, in_maps, list(range(NC_CORES)))
    out = np.concatenate(
        [res.results[c]["out"].reshape(-1, 1) for c in range(NC_CORES)], axis=0
    )
    return out.astype(np.float32)


# revision 14
# speedup vs baseline: 1.4216x; 1.0182x over previous
"""3-layer GCN (GCNConvNet) on 8 Trainium2 NeuronCores.

Math refactor: with isd = 1/sqrt(deg+1) and self-loop edges folded in,
each GCN layer  h' = relu( D^-1/2 (A+I) D^-1/2 (h W^T + 1 b^T) )  becomes

    g      = isd**2 * relu(Q_prev)          (node-major "source features")
    P[n]   = sum_{e: dst(e)=n} g[src(e)]    (+ g[n] self term)
    Q[n]   = Waug^T @ [P[n]; sigma[n]]      (Waug = [W^T; b], sigma = row sums)
    h'     = relu(isd * Q) = isd * relu(Q)  -> g' = isd^2 * relu(Q)

so every per-edge coefficient disappears into per-node scaling and the
scatter matrices are pure one-hot.  The final layer output is isd * Q2.

Sharding: nodes split into 8 contiguous dst ranges (6250 each).  Each core
computes P for its own range over ALL edges.  Edge gathers use
nc.gpsimd.dma_gather (int16 indices) with 256B elements that each cover
TWO unpadded 64-feature fp16 rows of the node-ordered g table; element
index = src//2 < 25000 fits int16 with no table permutation.  Edges are
bucketed by (dst tile, source-row parity) so each 128-edge chunk reads one
64-column half of its gathered window.

Layer 0's message windows are a pure permutation of host-known data
(g0 = isd*x), so they are prepacked on the host and loaded with plain
sync-engine DMAs -- zero GPSIMD descriptor generation for layer 0.  After
layers 0 and 1 a single AllGather (rank order == node order) rebuilds the
full g table.

The per-dst-tile chunk structure is derived from the actual edge data at
kernel() call time and padded to the max over the 8 cores so that all
cores run one shared NEFF (SPMD).
"""

import numpy as np

NC_CORES = 8
TILE = 128
GRP_TILES = 4  # dst tiles fused per PSUM/matmul group (4*128 = 512 <= max N)
D_F = 64  # feature width of hidden layers
WIN = 8  # chunks per gather window (8*128 = 1024 descs = SWDGE ring limit)
NQ = 4  # SWDGE queues


def _wrap16(v):
    """[S] int -> [128, S//16] int16, index i at [i%16, i//16], replicated x8."""
    S = v.shape[0]
    assert S % 16 == 0
    w = v.reshape(S // 16, 16).T.astype(np.int16)
    return np.ascontiguousarray(np.tile(w, (8, 1)))


def _prepare(x, edge_index, W0, b0, W1, b1, W2, b2):
    x = np.asarray(x, dtype=np.float32)
    ei = np.asarray(edge_index)
    W0 = np.asarray(W0, np.float32)
    b0 = np.asarray(b0, np.float32)
    W1 = np.asarray(W1, np.float32)
    b1 = np.asarray(b1, np.float32)
    W2 = np.asarray(W2, np.float32)
    b2 = np.asarray(b2, np.float32)

    N = x.shape[0]
    assert N % NC_CORES == 0
    OWN = N // NC_CORES
    assert OWN % 2 == 0
    ntiles = (OWN + TILE - 1) // TILE
    assert N // 2 <= 32768, "int16 element indices"
    src = ei[0].astype(np.int64)
    dst = ei[1].astype(np.int64)

    deg = np.bincount(dst, minlength=N).astype(np.float32) + 1.0
    isd = (1.0 / np.sqrt(deg)).astype(np.float32)
    sigma = (
        np.bincount(dst, weights=isd[src].astype(np.float64), minlength=N).astype(
            np.float32
        )
        + isd
    )

    g0 = (isd[:, None] * x).astype(np.float16)  # [N, 64] unpadded, node order

    # ---- edge bucketing: (core, tile, parity) -------------------------------
    par = src % 2
    elem = src // 2
    core = dst // OWN
    tl = (dst % OWN) // TILE
    key = ((core * ntiles + tl) * 2) + par
    order = np.argsort(key, kind="stable")
    s_elem = elem[order]
    s_dstl = (dst % OWN) % TILE
    s_dstl = s_dstl[order]
    counts = np.bincount(key, minlength=NC_CORES * ntiles * 2).reshape(
        NC_CORES, ntiles, 2
    )
    starts = np.zeros(NC_CORES * ntiles * 2 + 1, np.int64)
    np.cumsum(counts.reshape(-1), out=starts[1:])

    # chunks per (tile, parity), shared across cores
    C2 = np.maximum(1, -(-counts.max(axis=0) // TILE)).astype(np.int64)
    n_t = C2[:, 0] + C2[:, 1]

    a_off = np.zeros(ntiles + 1, np.int64)  # chunk offsets into the stream
    np.cumsum(n_t, out=a_off[1:])
    SA = int(a_off[-1]) * TILE
    nchunk = int(a_off[-1])

    per_core = []
    for c in range(NC_CORES):
        # pad slots must gather *something*; spread them over distinct
        # elements so they don't hammer one HBM line (S-col is -1 so the
        # gathered values never contribute).
        sA = np.arange(SA, dtype=np.int64) % (N // 2)
        dstl_flat = np.full(nchunk * TILE, -1.0, np.float32)
        for t in range(ntiles):
            slot = a_off[t] * TILE
            for p in range(2):
                k = (c * ntiles + t) * 2 + p
                lo, hi = starts[k], starts[k + 1]
                n = hi - lo
                sA[slot : slot + n] = s_elem[lo:hi]
                dstl_flat[slot : slot + n] = s_dstl[lo:hi]
                slot += C2[t, p] * TILE
        # layer-0 message windows: prepacked host-side (pure permutation)
        m0 = g0.reshape(N // 2, 2 * D_F)[sA]
        m0 = np.ascontiguousarray(
            m0.reshape(SA // TILE, TILE, 2 * D_F).transpose(1, 0, 2).reshape(
                TILE, SA
            )
        )
        own = isd[c * OWN : (c + 1) * OWN] ** 2
        tmp = np.zeros(ntiles * TILE, np.float32)
        tmp[:OWN] = own
        isd2 = np.ascontiguousarray(tmp.reshape(ntiles, TILE).T)
        per_core.append(
            dict(
                idxA=_wrap16(sA),
                msgs0=m0,
                dstl=np.ascontiguousarray(
                    dstl_flat.reshape(nchunk, TILE).T.astype(np.float16)
                ),
                sigma=sigma[c * OWN : (c + 1) * OWN]
                .astype(np.float16)
                .reshape(1, OWN),
                isd2=isd2,
                isdrow=isd[c * OWN : (c + 1) * OWN]
                .astype(np.float32)
                .reshape(1, OWN),
                g0own=np.ascontiguousarray(g0[c * OWN : (c + 1) * OWN]),
            )
        )

    waug = []
    for W, b in ((W0, b0), (W1, b1), (W2, b2)):
        wa = np.zeros((D_F + 1, W.shape[0]), np.float16)
        wa[:D_F, :] = W.T.astype(np.float16)
        wa[D_F, :] = b.astype(np.float16)
        waug.append(wa)

    iota = np.tile(np.arange(TILE, dtype=np.float16), (TILE, 1))
    ident = np.eye(TILE, dtype=np.float16)

    meta = dict(
        N=N,
        OWN=OWN,
        ntiles=ntiles,
        C2=C2,
        a_off=a_off,
        SA=SA,
        nchunk=nchunk,
        d_out=W2.shape[0],
    )

    in_maps = []
    for c in range(NC_CORES):
        m = dict(per_core[c])
        m["waug0"] = waug[0]
        m["waug1"] = waug[1]
        m["waug2"] = waug[2]
        m["iota"] = iota
        m["ident"] = ident
        in_maps.append(m)
    return meta, in_maps


def _build(meta, stage=99, n_dev=NC_CORES):
    import concourse.bacc as bacc
    import concourse.mybir as mybir
    from concourse.tile import TileContext

    f16 = mybir.dt.float16
    f32 = mybir.dt.float32
    i16 = mybir.dt.int16

    N = meta["N"]
    OWN = meta["OWN"]
    ntiles = meta["ntiles"]
    C2 = meta["C2"]
    a_off = meta["a_off"]
    SA, nchunk = meta["SA"], meta["nchunk"]
    d_out = meta["d_out"]

    ngrp = (ntiles + GRP_TILES - 1) // GRP_TILES
    grp_tiles = [
        list(range(g * GRP_TILES, min((g + 1) * GRP_TILES, ntiles)))
        for g in range(ngrp)
    ]
    max_ch = max(int(a_off[ts[-1] + 1] - a_off[ts[0]]) for ts in grp_tiles)

    nc = bacc.Bacc("TRN2", target_bir_lowering=False, num_devices=n_dev,
                  num_swdge_queues=NQ)

    msgs0_d = nc.dram_tensor("msgs0", [128, SA], f16, kind="ExternalInput")
    g0own_d = nc.dram_tensor("g0own", [OWN, D_F], f16, kind="ExternalInput")
    idxA_d = nc.dram_tensor("idxA", [128, SA // 16], i16, kind="ExternalInput")
    dstl_d = nc.dram_tensor("dstl", [128, nchunk], f16, kind="ExternalInput")
    waug_d = [
        nc.dram_tensor(f"waug{l}", [D_F + 1, do], f16, kind="ExternalInput")
        for l, do in enumerate([D_F, D_F, d_out])
    ]
    sigma_d = nc.dram_tensor("sigma", [1, OWN], f16, kind="ExternalInput")
    isd2_d = nc.dram_tensor("isd2", [TILE, ntiles], f32, kind="ExternalInput")
    isdrow_d = nc.dram_tensor("isdrow", [1, OWN], f32, kind="ExternalInput")
    iota_d = nc.dram_tensor("iota", [TILE, TILE], f16, kind="ExternalInput")
    ident_d = nc.dram_tensor("ident", [TILE, TILE], f16, kind="ExternalInput")
    out_d = nc.dram_tensor("out", [1, OWN], f32, kind="ExternalOutput")

    gown_d = [nc.dram_tensor(f"gown{l}", [OWN, D_F], f16) for l in (1, 2)]
    gfull_d = [
        nc.dram_tensor(f"gfull{l}", [N, D_F], f16, addr_space="Shared")
        for l in (1, 2)
    ]

    rg = [list(range(NC_CORES))]

    with TileContext(nc) as tc:
        with (
            tc.tile_pool(name="static", bufs=1) as stp,
            tc.tile_pool(name="msgs", bufs=10) as mp,
            tc.tile_pool(name="smat", bufs=4) as sp,
            tc.tile_pool(name="gself", bufs=3) as gp,
            tc.tile_pool(name="paug", bufs=3) as pp,
            tc.tile_pool(name="qrelu", bufs=3) as qp,
            tc.tile_pool(name="gout", bufs=3) as gop,
            tc.tile_pool(name="pps", bufs=3, space="PSUM") as p_ps,
            tc.tile_pool(name="qps", bufs=3, space="PSUM") as q_ps,
            tc.tile_pool(name="tps", bufs=2, space="PSUM") as t_ps,
        ):
            reg_cache = {}
            qn = [0]

            def nreg(v):
                if v not in reg_cache:
                    r = nc.gpsimd.alloc_register(f"nidx{v}")
                    nc.gpsimd.reg_mov(r, v)
                    reg_cache[v] = r
                return reg_cache[v]

            iota_sb = stp.tile([TILE, TILE], f16)
            nc.sync.dma_start(out=iota_sb[:], in_=iota_d[:])
            ident_sb = stp.tile([TILE, TILE], f16)
            nc.sync.dma_start(out=ident_sb[:], in_=ident_d[:])
            ident32_sb = stp.tile([TILE, TILE], f32)
            nc.vector.tensor_copy(ident32_sb[:], ident_sb[:])
            waug_sb = []
            for l, do in enumerate([D_F, D_F, d_out]):
                w = stp.tile([D_F + 1, do], f16, tag=f"waug{l}")
                nc.sync.dma_start(out=w[:], in_=waug_d[l][:])
                waug_sb.append(w)
            isd2_sb = stp.tile([TILE, ntiles], f32)
            nc.sync.dma_start(out=isd2_sb[:], in_=isd2_d[:])
            isdrow_sb = stp.tile([1, OWN], f32)
            nc.sync.dma_start(out=isdrow_sb[:], in_=isdrow_d[:])
            idxA_sb = stp.tile([128, SA // 16], i16)
            nc.sync.dma_start(out=idxA_sb[:], in_=idxA_d[:])
            dstl_sb = stp.tile([128, nchunk], f16)
            nc.sync.dma_start(out=dstl_sb[:], in_=dstl_d[:])
            out_sb = stp.tile([1, OWN], f32)

            nch_all = SA // TILE

            def emit_windows_gather(gtab):
                gslab = gtab[0:N, :].rearrange("(a b) f -> a (b f)", b=2)
                lst = []
                for w in range(0, nch_all, WIN):
                    kw = min(WIN, nch_all - w)
                    wt = mp.tile([128, WIN * TILE], f16, tag="win")
                    nc.gpsimd.dma_gather(
                        wt[:, : kw * TILE].rearrange("p (c e) -> p c e", e=TILE),
                        gslab,
                        idxA_sb[:, w * 8 : (w + kw) * 8],
                        kw * TILE,
                        nreg(kw * TILE),
                        TILE,
                        queue_num=qn[0],
                    )
                    qn[0] = (qn[0] + 1) % NQ
                    lst.append(wt)
                return lst

            def emit_windows_dram():
                # layer 0: plain HWDGE loads of host-prepacked messages
                lst = []
                for w in range(0, nch_all, WIN):
                    kw = min(WIN, nch_all - w)
                    wt = mp.tile([128, WIN * TILE], f16, tag="win")
                    nc.sync.dma_start(
                        out=wt[:, : kw * TILE],
                        in_=msgs0_d[:, w * TILE : (w + kw) * TILE],
                    )
                    lst.append(wt)
                return lst

            nlayers = 3 if stage >= 7 else 1
            if stage < 7:
                nc.vector.memset(out_sb[:], 0.0)
            wins = emit_windows_dram()
            for layer in range(nlayers):
                gown_src = [g0own_d, gown_d[0], gown_d[1]][layer]
                do = D_F if layer < 2 else d_out

                def msg_lhs(chunk, parity):
                    wt = wins[chunk // WIN]
                    col = (chunk % WIN) * TILE + parity * D_F
                    return wt[:, col : col + D_F]

                for g, ts in enumerate(grp_tiles):
                    t0, t1 = ts[0], ts[-1] + 1
                    gw = (t1 - t0) * TILE
                    row0 = t0 * TILE
                    rows = min(gw, OWN - row0)
                    c0 = int(a_off[t0])
                    nch = int(a_off[t1] - c0)

                    if stage < 2:
                        continue
                    S = sp.tile([128, max_ch * TILE], f16, tag="S")
                    nc.vector.tensor_tensor(
                        S[:, : nch * TILE].rearrange("p (c e) -> p c e", e=TILE),
                        iota_sb[:].unsqueeze(1).broadcast_to([TILE, nch, TILE]),
                        dstl_sb[:, c0 : c0 + nch]
                        .unsqueeze(2)
                        .broadcast_to([TILE, nch, TILE]),
                        mybir.AluOpType.is_equal,
                    )

                    if stage < 3:
                        continue
                    gself = gp.tile([TILE, (t1 - t0) * D_F], f16, tag="gself")
                    if rows < gw:
                        nc.vector.memset(gself[:], 0.0)
                    for ti, t in enumerate(ts):
                        r0 = row0 + ti * TILE
                        r = min(TILE, OWN - r0)
                        nc.sync.dma_start(
                            out=gself[0:r, ti * D_F : ti * D_F + D_F],
                            in_=gown_src[r0 : r0 + r, :],
                        )

                    ps = p_ps.tile([D_F, gw], f32, space="PSUM", tag="ps")
                    for ti, t in enumerate(ts):
                        sl = slice(ti * TILE, (ti + 1) * TILE)
                        n0, n1 = int(C2[t, 0]), int(C2[t, 1])
                        nmm = n0 + n1
                        nc.tensor.matmul(
                            out=ps[:, sl],
                            lhsT=gself[:, ti * D_F : ti * D_F + D_F],
                            rhs=ident_sb[:],
                            start=True,
                            stop=(nmm == 0),
                        )
                        for j in range(nmm):
                            par = int(j >= n0)
                            lhs = msg_lhs(int(a_off[t]) + j, par)
                            scol = (int(a_off[t]) - c0 + j) * TILE
                            nc.tensor.matmul(
                                out=ps[:, sl],
                                lhsT=lhs,
                                rhs=S[:, scol : scol + TILE],
                                start=False,
                                stop=(j == nmm - 1),
                            )

                    if stage < 4:
                        continue
                    paug = pp.tile([D_F + 1, gw], f16, tag="paug")
                    nc.vector.tensor_copy(paug[0:D_F, :gw], ps[:, :gw])
                    nc.sync.dma_start(
                        out=paug[D_F : D_F + 1, 0:rows],
                        in_=sigma_d[:, row0 : row0 + rows],
                    )
                    if rows < gw:
                        nc.vector.memset(paug[D_F : D_F + 1, rows:gw], 0.0)
                    qs = q_ps.tile([D_F, gw], f32, space="PSUM", tag="qs")
                    nc.tensor.matmul(
                        out=qs[0:do, :gw],
                        lhsT=waug_sb[layer][:],
                        rhs=paug[:, :gw],
                        start=True,
                        stop=True,
                    )

                    if stage < 5:
                        continue
                    if layer < 2:
                        qr = qp.tile([D_F, gw], f32, tag="qr")
                        nc.scalar.activation(
                            qr[:, :gw],
                            qs[0:D_F, :gw],
                            mybir.ActivationFunctionType.Relu,
                        )
                        for ti, t in enumerate(ts):
                            qt = t_ps.tile([TILE, D_F], f32, space="PSUM", tag="qt")
                            nc.tensor.transpose(
                                out=qt[:],
                                in_=qr[:, ti * TILE : (ti + 1) * TILE],
                                identity=ident32_sb[0:D_F, 0:D_F],
                            )
                            gsl = gop.tile([TILE, D_F], f16, tag="gsl")
                            nc.vector.tensor_scalar_mul(
                                gsl[:], qt[:], isd2_sb[:, t : t + 1]
                            )
                            r0 = row0 + ti * TILE
                            r = min(TILE, OWN - r0)
                            nc.sync.dma_start(
                                out=gown_d[layer][r0 : r0 + r, :],
                                in_=gsl[0:r, :],
                            )
                    else:
                        nc.vector.tensor_copy(
                            out_sb[:, row0 : row0 + rows], qs[0:1, 0:rows]
                        )

                if layer < 2 and stage >= 6 and stage != 8:
                    nc.gpsimd.collective_compute(
                        "AllGather",
                        mybir.AluOpType.bypass,
                        replica_groups=rg,
                        ins=[gown_d[layer][:]],
                        outs=[gfull_d[layer][:]],
                    )
                    wins = emit_windows_gather(gfull_d[layer])

            nc.vector.tensor_tensor(
                out_sb[:], out_sb[:], isdrow_sb[:], mybir.AluOpType.mult
            )
            nc.sync.dma_start(out=out_d[:], in_=out_sb[:])

    nc.compile()
    return nc


def kernel(x, edge_index, W0, b0, W1, b1, W2, b2):
    from concourse.bass_utils import run_bass_kernel_spmd

    meta, in_maps = _prepare(x, edge_index, W0, b0, W1, b1, W2, b2)
    nc = _build(meta)
    res = run_bass_kernel_spmd(nc, in_maps, list(range(NC_CORES)))
    out = np.concatenate(
        [res.results[c]["out"].reshape(-1, 1) for c in range(NC_CORES)], axis=0
    )
    return out.astype(np.float32)


# revision 15
# speedup vs baseline: 1.4551x; 1.0236x over previous
"""3-layer GCN (GCNConvNet) on 8 Trainium2 NeuronCores.

Math refactor: with isd = 1/sqrt(deg+1) and self-loop edges folded in,
each GCN layer  h' = relu( D^-1/2 (A+I) D^-1/2 (h W^T + 1 b^T) )  becomes

    g      = isd**2 * relu(Q_prev)          (node-major "source features")
    P[n]   = sum_{e: dst(e)=n} g[src(e)]    (+ g[n] self term)
    Q[n]   = Waug^T @ [P[n]; sigma[n]]      (Waug = [W^T; b], sigma = row sums)
    h'     = relu(isd * Q) = isd * relu(Q)  -> g' = isd^2 * relu(Q)

so every per-edge coefficient disappears into per-node scaling and the
scatter matrices are pure one-hot.  The final layer output is isd * Q2.

Sharding: nodes split into 8 contiguous dst ranges (6250 each).  Each core
computes P for its own range over ALL edges.  Edge gathers use
nc.gpsimd.dma_gather (int16 indices) with 256B elements that each cover
TWO unpadded 64-feature fp16 rows of the node-ordered g table; element
index = src//2 < 25000 fits int16 with no table permutation.  Edges are
bucketed by (dst tile, source-row parity) so each 128-edge chunk reads one
64-column half of its gathered window.

Layer 0's message windows are a pure permutation of host-known data
(g0 = isd*x), so they are prepacked on the host and loaded with plain
sync-engine DMAs -- zero GPSIMD descriptor generation for layer 0.  After
layers 0 and 1 a single AllGather (rank order == node order) rebuilds the
full g table.

The per-dst-tile chunk structure is derived from the actual edge data at
kernel() call time and padded to the max over the 8 cores so that all
cores run one shared NEFF (SPMD).
"""

import numpy as np

NC_CORES = 8
TILE = 128
GRP_TILES = 4  # dst tiles fused per PSUM/matmul group (4*128 = 512 <= max N)
D_F = 64  # feature width of hidden layers
WIN = 8  # chunks per gather window (8*128 = 1024 descs = SWDGE ring limit)
NQ = 4  # SWDGE queues


def _wrap16(v):
    """[S] int -> [128, S//16] int16, index i at [i%16, i//16], replicated x8."""
    S = v.shape[0]
    assert S % 16 == 0
    w = v.reshape(S // 16, 16).T.astype(np.int16)
    return np.ascontiguousarray(np.tile(w, (8, 1)))


def _prepare(x, edge_index, W0, b0, W1, b1, W2, b2):
    x = np.asarray(x, dtype=np.float32)
    ei = np.asarray(edge_index)
    W0 = np.asarray(W0, np.float32)
    b0 = np.asarray(b0, np.float32)
    W1 = np.asarray(W1, np.float32)
    b1 = np.asarray(b1, np.float32)
    W2 = np.asarray(W2, np.float32)
    b2 = np.asarray(b2, np.float32)

    N = x.shape[0]
    assert N % NC_CORES == 0
    OWN = N // NC_CORES
    assert OWN % 2 == 0
    ntiles = (OWN + TILE - 1) // TILE
    assert N // 2 <= 32768, "int16 element indices"
    src = ei[0].astype(np.int64)
    dst = ei[1].astype(np.int64)

    deg = np.bincount(dst, minlength=N).astype(np.float32) + 1.0
    isd = (1.0 / np.sqrt(deg)).astype(np.float32)
    sigma = (
        np.bincount(dst, weights=isd[src].astype(np.float64), minlength=N).astype(
            np.float32
        )
        + isd
    )

    g0 = (isd[:, None] * x).astype(np.float16)  # [N, 64] unpadded, node order

    # ---- edge bucketing: (core, tile, parity) -------------------------------
    par = src % 2
    elem = src // 2
    core = dst // OWN
    tl = (dst % OWN) // TILE
    key = ((core * ntiles + tl) * 2) + par
    order = np.argsort(key, kind="stable")
    s_elem = elem[order]
    s_dstl = (dst % OWN) % TILE
    s_dstl = s_dstl[order]
    counts = np.bincount(key, minlength=NC_CORES * ntiles * 2).reshape(
        NC_CORES, ntiles, 2
    )
    starts = np.zeros(NC_CORES * ntiles * 2 + 1, np.int64)
    np.cumsum(counts.reshape(-1), out=starts[1:])

    # chunks per (tile, parity), shared across cores
    C2 = np.maximum(1, -(-counts.max(axis=0) // TILE)).astype(np.int64)
    n_t = C2[:, 0] + C2[:, 1]

    a_off = np.zeros(ntiles + 1, np.int64)  # chunk offsets into the stream
    np.cumsum(n_t, out=a_off[1:])
    SA = int(a_off[-1]) * TILE
    nchunk = int(a_off[-1])

    per_core = []
    for c in range(NC_CORES):
        # pad slots must gather *something*; spread them over distinct
        # elements so they don't hammer one HBM line (S-col is -1 so the
        # gathered values never contribute).
        sA = np.arange(SA, dtype=np.int64) % (N // 2)
        dstl_flat = np.full(nchunk * TILE, -1.0, np.float32)
        for t in range(ntiles):
            slot = a_off[t] * TILE
            for p in range(2):
                k = (c * ntiles + t) * 2 + p
                lo, hi = starts[k], starts[k + 1]
                n = hi - lo
                sA[slot : slot + n] = s_elem[lo:hi]
                dstl_flat[slot : slot + n] = s_dstl[lo:hi]
                slot += C2[t, p] * TILE
        # layer-0 message windows: prepacked host-side (pure permutation)
        m0 = g0.reshape(N // 2, 2 * D_F)[sA]
        m0 = np.ascontiguousarray(
            m0.reshape(SA // TILE, TILE, 2 * D_F).transpose(1, 0, 2).reshape(
                TILE, SA
            )
        )
        own = isd[c * OWN : (c + 1) * OWN] ** 2
        tmp = np.zeros(ntiles * TILE, np.float32)
        tmp[:OWN] = own
        isd2 = np.ascontiguousarray(tmp.reshape(ntiles, TILE).T)
        per_core.append(
            dict(
                idxA=_wrap16(sA),
                msgs0=m0,
                dstl=np.ascontiguousarray(
                    dstl_flat.reshape(nchunk, TILE).T.astype(np.float16)
                ),
                sigma=sigma[c * OWN : (c + 1) * OWN]
                .astype(np.float16)
                .reshape(1, OWN),
                isd2=isd2,
                isdrow=isd[c * OWN : (c + 1) * OWN]
                .astype(np.float32)
                .reshape(1, OWN),
                g0own=np.ascontiguousarray(g0[c * OWN : (c + 1) * OWN]),
            )
        )

    waug = []
    for W, b in ((W0, b0), (W1, b1), (W2, b2)):
        wa = np.zeros((D_F + 1, W.shape[0]), np.float16)
        wa[:D_F, :] = W.T.astype(np.float16)
        wa[D_F, :] = b.astype(np.float16)
        waug.append(wa)

    iota = np.tile(np.arange(TILE, dtype=np.float16), (TILE, 1))
    ident = np.eye(TILE, dtype=np.float16)

    meta = dict(
        N=N,
        OWN=OWN,
        ntiles=ntiles,
        C2=C2,
        a_off=a_off,
        SA=SA,
        nchunk=nchunk,
        d_out=W2.shape[0],
    )

    in_maps = []
    for c in range(NC_CORES):
        m = dict(per_core[c])
        m["waug0"] = waug[0]
        m["waug1"] = waug[1]
        m["waug2"] = waug[2]
        m["iota"] = iota
        m["ident"] = ident
        in_maps.append(m)
    return meta, in_maps


def _build(meta, stage=99, n_dev=NC_CORES):
    import concourse.bacc as bacc
    import concourse.mybir as mybir
    from concourse.tile import TileContext

    f16 = mybir.dt.float16
    f32 = mybir.dt.float32
    i16 = mybir.dt.int16

    N = meta["N"]
    OWN = meta["OWN"]
    ntiles = meta["ntiles"]
    C2 = meta["C2"]
    a_off = meta["a_off"]
    SA, nchunk = meta["SA"], meta["nchunk"]
    d_out = meta["d_out"]

    ngrp = (ntiles + GRP_TILES - 1) // GRP_TILES
    grp_tiles = [
        list(range(g * GRP_TILES, min((g + 1) * GRP_TILES, ntiles)))
        for g in range(ngrp)
    ]
    max_ch = max(int(a_off[ts[-1] + 1] - a_off[ts[0]]) for ts in grp_tiles)

    nc = bacc.Bacc("TRN2", target_bir_lowering=False, num_devices=n_dev,
                  num_swdge_queues=NQ)

    msgs0_d = nc.dram_tensor("msgs0", [128, SA], f16, kind="ExternalInput")
    g0own_d = nc.dram_tensor("g0own", [OWN, D_F], f16, kind="ExternalInput")
    idxA_d = nc.dram_tensor("idxA", [128, SA // 16], i16, kind="ExternalInput")
    dstl_d = nc.dram_tensor("dstl", [128, nchunk], f16, kind="ExternalInput")
    waug_d = [
        nc.dram_tensor(f"waug{l}", [D_F + 1, do], f16, kind="ExternalInput")
        for l, do in enumerate([D_F, D_F, d_out])
    ]
    sigma_d = nc.dram_tensor("sigma", [1, OWN], f16, kind="ExternalInput")
    isd2_d = nc.dram_tensor("isd2", [TILE, ntiles], f32, kind="ExternalInput")
    isdrow_d = nc.dram_tensor("isdrow", [1, OWN], f32, kind="ExternalInput")
    iota_d = nc.dram_tensor("iota", [TILE, TILE], f16, kind="ExternalInput")
    ident_d = nc.dram_tensor("ident", [TILE, TILE], f16, kind="ExternalInput")
    out_d = nc.dram_tensor("out", [1, OWN], f32, kind="ExternalOutput")

    gown_d = [nc.dram_tensor(f"gown{l}", [OWN, D_F], f16) for l in (1, 2)]
    gfull_d = [
        nc.dram_tensor(f"gfull{l}", [N, D_F], f16, addr_space="Shared")
        for l in (1, 2)
    ]

    rg = [list(range(NC_CORES))]

    with TileContext(nc) as tc:
        with (
            tc.tile_pool(name="static", bufs=1) as stp,
            tc.tile_pool(name="msgs", bufs=10) as mp,
            tc.tile_pool(name="smat", bufs=4) as sp,
            tc.tile_pool(name="gself", bufs=3) as gp,
            tc.tile_pool(name="paug", bufs=3) as pp,
            tc.tile_pool(name="qrelu", bufs=3) as qp,
            tc.tile_pool(name="gout", bufs=4) as gop,
            tc.tile_pool(name="pps", bufs=3, space="PSUM") as p_ps,
            tc.tile_pool(name="qps", bufs=3, space="PSUM") as q_ps,
            tc.tile_pool(name="tps", bufs=2, space="PSUM") as t_ps,
        ):
            reg_cache = {}
            qn = [0]

            def nreg(v):
                if v not in reg_cache:
                    r = nc.gpsimd.alloc_register(f"nidx{v}")
                    nc.gpsimd.reg_mov(r, v)
                    reg_cache[v] = r
                return reg_cache[v]

            iota_sb = stp.tile([TILE, TILE], f16)
            nc.sync.dma_start(out=iota_sb[:], in_=iota_d[:])
            ident_sb = stp.tile([TILE, TILE], f16)
            nc.sync.dma_start(out=ident_sb[:], in_=ident_d[:])
            ident32_sb = stp.tile([TILE, TILE], f32)
            nc.vector.tensor_copy(ident32_sb[:], ident_sb[:])
            waug_sb = []
            for l, do in enumerate([D_F, D_F, d_out]):
                w = stp.tile([D_F + 1, do], f16, tag=f"waug{l}")
                nc.sync.dma_start(out=w[:], in_=waug_d[l][:])
                waug_sb.append(w)
            isd2_sb = stp.tile([TILE, ntiles], f32)
            nc.sync.dma_start(out=isd2_sb[:], in_=isd2_d[:])
            isdrow_sb = stp.tile([1, OWN], f32)
            nc.sync.dma_start(out=isdrow_sb[:], in_=isdrow_d[:])
            idxA_sb = stp.tile([128, SA // 16], i16)
            nc.sync.dma_start(out=idxA_sb[:], in_=idxA_d[:])
            dstl_sb = stp.tile([128, nchunk], f16)
            nc.sync.dma_start(out=dstl_sb[:], in_=dstl_d[:])
            out_sb = stp.tile([1, OWN], f32)

            nch_all = SA // TILE

            def emit_windows_gather(gtab):
                gslab = gtab[0:N, :].rearrange("(a b) f -> a (b f)", b=2)
                lst = []
                for w in range(0, nch_all, WIN):
                    kw = min(WIN, nch_all - w)
                    wt = mp.tile([128, WIN * TILE], f16, tag="win")
                    nc.gpsimd.dma_gather(
                        wt[:, : kw * TILE].rearrange("p (c e) -> p c e", e=TILE),
                        gslab,
                        idxA_sb[:, w * 8 : (w + kw) * 8],
                        kw * TILE,
                        nreg(kw * TILE),
                        TILE,
                        queue_num=qn[0],
                    )
                    qn[0] = (qn[0] + 1) % NQ
                    lst.append(wt)
                return lst

            def emit_windows_dram():
                # layer 0: plain HWDGE loads of host-prepacked messages
                lst = []
                for w in range(0, nch_all, WIN):
                    kw = min(WIN, nch_all - w)
                    wt = mp.tile([128, WIN * TILE], f16, tag="win")
                    nc.sync.dma_start(
                        out=wt[:, : kw * TILE],
                        in_=msgs0_d[:, w * TILE : (w + kw) * TILE],
                    )
                    lst.append(wt)
                return lst

            nlayers = 3 if stage >= 7 else 1
            if stage < 7:
                nc.vector.memset(out_sb[:], 0.0)
            wins = emit_windows_dram()
            for layer in range(nlayers):
                gown_src = [g0own_d, gown_d[0], gown_d[1]][layer]
                do = D_F if layer < 2 else d_out

                def msg_lhs(chunk, parity):
                    wt = wins[chunk // WIN]
                    col = (chunk % WIN) * TILE + parity * D_F
                    return wt[:, col : col + D_F]

                for g, ts in enumerate(grp_tiles):
                    t0, t1 = ts[0], ts[-1] + 1
                    gw = (t1 - t0) * TILE
                    row0 = t0 * TILE
                    rows = min(gw, OWN - row0)
                    c0 = int(a_off[t0])
                    nch = int(a_off[t1] - c0)

                    if stage < 2:
                        continue
                    S = sp.tile([128, max_ch * TILE], f16, tag="S")
                    nc.vector.tensor_tensor(
                        S[:, : nch * TILE].rearrange("p (c e) -> p c e", e=TILE),
                        iota_sb[:].unsqueeze(1).broadcast_to([TILE, nch, TILE]),
                        dstl_sb[:, c0 : c0 + nch]
                        .unsqueeze(2)
                        .broadcast_to([TILE, nch, TILE]),
                        mybir.AluOpType.is_equal,
                    )

                    if stage < 3:
                        continue
                    gself = gp.tile([TILE, (t1 - t0) * D_F], f16, tag="gself")
                    if rows < gw:
                        nc.vector.memset(gself[:], 0.0)
                    for ti, t in enumerate(ts):
                        r0 = row0 + ti * TILE
                        r = min(TILE, OWN - r0)
                        nc.sync.dma_start(
                            out=gself[0:r, ti * D_F : ti * D_F + D_F],
                            in_=gown_src[r0 : r0 + r, :],
                        )

                    ps = p_ps.tile([D_F, gw], f32, space="PSUM", tag="ps")
                    for ti, t in enumerate(ts):
                        sl = slice(ti * TILE, (ti + 1) * TILE)
                        n0, n1 = int(C2[t, 0]), int(C2[t, 1])
                        nmm = n0 + n1
                        nc.tensor.matmul(
                            out=ps[:, sl],
                            lhsT=gself[:, ti * D_F : ti * D_F + D_F],
                            rhs=ident_sb[:],
                            start=True,
                            stop=(nmm == 0),
                        )
                        for j in range(nmm):
                            par = int(j >= n0)
                            lhs = msg_lhs(int(a_off[t]) + j, par)
                            scol = (int(a_off[t]) - c0 + j) * TILE
                            nc.tensor.matmul(
                                out=ps[:, sl],
                                lhsT=lhs,
                                rhs=S[:, scol : scol + TILE],
                                start=False,
                                stop=(j == nmm - 1),
                            )

                    if stage < 4:
                        continue
                    paug = pp.tile([D_F + 1, gw], f16, tag="paug")
                    nc.scalar.activation(
                        paug[0:D_F, :gw],
                        ps[:, :gw],
                        mybir.ActivationFunctionType.Copy,
                    )
                    nc.sync.dma_start(
                        out=paug[D_F : D_F + 1, 0:rows],
                        in_=sigma_d[:, row0 : row0 + rows],
                    )
                    if rows < gw:
                        nc.vector.memset(paug[D_F : D_F + 1, rows:gw], 0.0)
                    qs = q_ps.tile([D_F, gw], f32, space="PSUM", tag="qs")
                    nc.tensor.matmul(
                        out=qs[0:do, :gw],
                        lhsT=waug_sb[layer][:],
                        rhs=paug[:, :gw],
                        start=True,
                        stop=True,
                    )

                    if stage < 5:
                        continue
                    if layer < 2:
                        qr = qp.tile([D_F, gw], f32, tag="qr")
                        nc.scalar.activation(
                            qr[:, :gw],
                            qs[0:D_F, :gw],
                            mybir.ActivationFunctionType.Relu,
                        )
                        for ti, t in enumerate(ts):
                            qt = t_ps.tile([TILE, D_F], f32, space="PSUM", tag="qt")
                            nc.tensor.transpose(
                                out=qt[:],
                                in_=qr[:, ti * TILE : (ti + 1) * TILE],
                                identity=ident32_sb[0:D_F, 0:D_F],
                            )
                            gsl = gop.tile([TILE, D_F], f16, tag="gsl")
                            nc.scalar.activation(
                                gsl[:],
                                qt[:],
                                mybir.ActivationFunctionType.Copy,
                                scale=isd2_sb[:, t : t + 1],
                            )
                            r0 = row0 + ti * TILE
                            r = min(TILE, OWN - r0)
                            nc.sync.dma_start(
                                out=gown_d[layer][r0 : r0 + r, :],
                                in_=gsl[0:r, :],
                            )
                    else:
                        nc.vector.tensor_copy(
                            out_sb[:, row0 : row0 + rows], qs[0:1, 0:rows]
                        )

                if layer < 2 and stage >= 6 and stage != 8:
                    nc.gpsimd.collective_compute(
                        "AllGather",
                        mybir.AluOpType.bypass,
                        replica_groups=rg,
                        ins=[gown_d[layer][:]],
                        outs=[gfull_d[layer][:]],
                    )
                    wins = emit_windows_gather(gfull_d[layer])

            nc.vector.tensor_tensor(
                out_sb[:], out_sb[:], isdrow_sb[:], mybir.AluOpType.mult
            )
            nc.sync.dma_start(out=out_d[:], in_=out_sb[:])

    nc.compile()
    return nc


def kernel(x, edge_index, W0, b0, W1, b1, W2, b2):
    from concourse.bass_utils import run_bass_kernel_spmd

    meta, in_maps = _prepare(x, edge_index, W0, b0, W1, b1, W2, b2)
    nc = _build(meta)
    res = run_bass_kernel_spmd(nc, in_maps, list(range(NC_CORES)))
    out = np.concatenate(
        [res.results[c]["out"].reshape(-1, 1) for c in range(NC_CORES)], axis=0
    )
    return out.astype(np.float32)
